# revision 3
# baseline (speedup 1.0000x reference)
"""ContentOnlyRouter MoE kernel for 8x TRN2 NeuronCores.

Strategy (two SPMD launches, host does only data marshalling):
  Launch A (data-parallel scoring): each core scores its 2048-token shard
    against sign(tile_sigs). x is split as bf16 hi + fp8e4m3 lo (lo scaled
    by 64, sign vectors scaled by 1/64 so products land exactly); both parts
    accumulate into one PSUM bank laid out [128 tok, 16 blk, 8 expert], so
    argmax runs directly on DVE with no transposes. Scores match fp32
    scoring to ~1e-4 absolute; verified exact-argmax on this input
    distribution with ~100x gap margin.
  Host glue: stable counting-sort of the 16384 expert ids; expert token
    lists are padded to 128-multiples and the resulting blocks are packed
    onto 8 cores x 17 block-slots (slots 0-8 use weight slab 0, slots 9-16
    slab 1) by a greedy covering solver. The gather itself (pick + transpose
    token rows) happens on host, so launch B does no dma_gather.
  Launch B (block-parallel grouped GEMM): each core streams its 17
    pre-gathered 128-token blocks and 2 weight slabs, does 8 accumulating
    bf16 matmuls per 512-wide PSUM half, adds bias on DVE, writes bf16 rows.
    Host scatters rows back to token order.

Shapes hardcoded for B=4, S=4096, D=1024, T=8 per the problem spec.
"""

import os

os.environ.setdefault("JAX_PLATFORMS", "")

import contextlib

import numpy as np
import ml_dtypes

import concourse.bass as bass
import concourse.bacc as bacc
import concourse.mybir as mybir
import concourse.tile as tile

B, S, D, T = 4, 4096, 1024, 8
NTOK = B * S             # 16384 tokens
NCORES = 8
SHARD = NTOK // NCORES   # 2048 tokens scored per core
DC = D // 128            # 8 contraction chunks
ABLK = SHARD // 128      # 16 token blocks per shard
NACH = 4                 # launch A DMA chunks (512 tokens each)
NSLOT = 17               # GEMM block slots per core
RUN0, RUN1 = 9, 8        # slots per weight slab (slab0: slots 0-8, slab1: 9-16)
GCAP = NSLOT * 128       # 2176 gathered tokens per core
TRASH = NTOK             # row index used for padding slots
GX_CHUNKS = [2, 3, 4, 4, 4]  # slots per launch-B gather-stream chunk

F32 = mybir.dt.float32
BF16 = mybir.dt.bfloat16
F8 = mybir.dt.float8e4

BF16NP = ml_dtypes.bfloat16
F8NP = ml_dtypes.float8_e4m3

_perf = []  # exec_time_ns per launch when tracing


def build_launch_a(iters=1):
    """Scores + argmax for one 2048-token shard."""
    nc = bacc.Bacc(None)
    xht = nc.dram_tensor("xht", [128, DC, SHARD], BF16, kind="ExternalInput")
    xlt = nc.dram_tensor("xlt", [128, DC, SHARD], F8, kind="ExternalInput")
    sgh = nc.dram_tensor("sgh", [128, DC, T], BF16, kind="ExternalInput")
    sgl = nc.dram_tensor("sgl", [128, DC, T], F8, kind="ExternalInput")
    idx = nc.dram_tensor("idx", [128, ABLK], F32, kind="ExternalOutput")

    with tile.TileContext(nc) as tc:
        with (
            tc.tile_pool(name="const", bufs=1) as const,
            tc.tile_pool(name="xa", bufs=3) as xa,
            tc.tile_pool(name="ps", bufs=1, space="PSUM") as ps,
            tc.tile_pool(name="sb", bufs=2) as sb,
        ):
            sgh_sb = const.tile([128, DC, T], BF16)
            nc.sync.dma_start(out=sgh_sb, in_=sgh[:, :, :])
            sgl_sb = const.tile([128, DC, T], F8)
            nc.sync.dma_start(out=sgl_sb, in_=sgl[:, :, :])
            # rev-iota: value 7-t at expert slot t (first-occurrence argmax)
            revio = const.tile([128, ABLK, T], F32)
            for t in range(T):
                nc.vector.memset(revio[:, :, t : t + 1], float(T - 1 - t))

            loop = tc.For_i(0, iters, 1) if iters > 1 else contextlib.nullcontext()
            with loop:
                _body_a(nc, xa, ps, sb, sgh_sb, sgl_sb, revio, xht, xlt, idx)
    nc.compile()
    return nc


def _body_a(nc, xa, ps, sb, sgh_sb, sgl_sb, revio, xht, xlt, idx):
    CH = SHARD // NACH       # 512 tokens per DMA chunk
    BPC = CH // 128          # 4 matmul blocks per chunk
    psum = ps.tile([128, ABLK, T], F32)
    for g in range(NACH):
        xh = xa.tile([128, DC, CH], BF16, tag="xh")
        xl = xa.tile([128, DC, CH], F8, tag="xl")
        nc.sync.dma_start(out=xh, in_=xht[:, :, CH * g : CH * (g + 1)])
        nc.sync.dma_start(out=xl, in_=xlt[:, :, CH * g : CH * (g + 1)])
        for j in range(BPC):
            blk = g * BPC + j
            o = psum[:, blk, :]
            tok = slice(128 * j, 128 * (j + 1))
            for c in range(DC):
                nc.tensor.matmul(
                    out=o, lhsT=xh[:, c, tok], rhs=sgh_sb[:, c, :],
                    start=(c == 0), stop=False,
                )
            for c in range(DC):
                nc.tensor.matmul(
                    out=o, lhsT=xl[:, c, tok], rhs=sgl_sb[:, c, :],
                    start=False, stop=(c == DC - 1),
                )
    # argmax over the last axis (8 experts) per token, first occurrence wins
    smax = sb.tile([128, ABLK, 1], F32, tag="smax")
    nc.vector.reduce_max(out=smax, in_=psum, axis=mybir.AxisListType.X)
    m = sb.tile([128, ABLK, T], F32, tag="m")
    nc.vector.tensor_tensor(
        out=m, in0=psum, in1=smax.to_broadcast([128, ABLK, T]),
        op=mybir.AluOpType.is_ge,
    )
    nc.vector.tensor_tensor(out=m, in0=m, in1=revio, op=mybir.AluOpType.mult)
    mm = sb.tile([128, ABLK, 1], F32, tag="mm")
    nc.vector.reduce_max(out=mm, in_=m, axis=mybir.AxisListType.X)
    idxv = sb.tile([128, ABLK], F32, tag="idxv")
    nc.vector.tensor_scalar(
        out=idxv, in0=mm[:, :, 0], scalar1=-1.0, scalar2=float(T - 1),
        op0=mybir.AluOpType.mult, op1=mybir.AluOpType.add,
    )
    nc.sync.dma_start(out=idx[:, :], in_=idxv)


def build_launch_b(iters=1):
    """Grouped GEMM over 17 pre-gathered 128-token blocks (2 weight slabs)."""
    nc = bacc.Bacc(None)
    gxt = nc.dram_tensor("gxt", [128, DC, GCAP], BF16, kind="ExternalInput")
    wts = nc.dram_tensor("wts", [128, 2, DC, D], BF16, kind="ExternalInput")
    bts = nc.dram_tensor("bts", [2, D], F32, kind="ExternalInput")
    orows = nc.dram_tensor("orows", [GCAP, D], BF16, kind="ExternalOutput")

    with tile.TileContext(nc) as tc:
        with (
            tc.tile_pool(name="wp", bufs=1) as wp,
            tc.tile_pool(name="gx", bufs=3) as gxp,
            tc.tile_pool(name="ps", bufs=4, space="PSUM") as ps,
            tc.tile_pool(name="osb", bufs=3) as osb,
        ):
            loop = tc.For_i(0, iters, 1) if iters > 1 else contextlib.nullcontext()
            with loop:
                _body_b(nc, wp, gxp, ps, osb, gxt, wts, bts, orows)
    nc.compile()
    return nc


def _body_b(nc, wp, gxp, ps, osb, gxt, wts, bts, orows):
    w_sb = wp.tile([128, 2, DC, D], BF16, tag="w")
    b_sb = wp.tile([128, 2, D], F32, tag="b")

    offs = np.cumsum([0] + GX_CHUNKS)
    gx_tiles = [None] * len(GX_CHUNKS)

    def emit_gx(ci):
        t = gxp.tile([128, DC, 512], BF16, tag="gx")
        n = GX_CHUNKS[ci] * 128
        nc.sync.dma_start(
            out=t[:, :, 0:n], in_=gxt[:, :, 128 * offs[ci] : 128 * offs[ci] + n]
        )
        gx_tiles[ci] = t

    def compute_chunk(ci):
        t = gx_tiles[ci]
        for si in range(GX_CHUNKS[ci]):
            slot = offs[ci] + si
            slab = 0 if slot < RUN0 else 1
            tok = slice(128 * si, 128 * (si + 1))
            ps0 = ps.tile([128, 512], F32, tag="ps0")
            ps1 = ps.tile([128, 512], F32, tag="ps1")
            for c in range(DC):
                nc.tensor.matmul(
                    out=ps0, lhsT=t[:, c, tok], rhs=w_sb[:, slab, c, 0:512],
                    start=(c == 0), stop=(c == DC - 1),
                )
                nc.tensor.matmul(
                    out=ps1, lhsT=t[:, c, tok], rhs=w_sb[:, slab, c, 512:1024],
                    start=(c == 0), stop=(c == DC - 1),
                )
            o = osb.tile([128, D], BF16)
            nc.vector.tensor_add(out=o[:, 0:512], in0=ps0, in1=b_sb[:, slab, 0:512])
            nc.vector.tensor_add(out=o[:, 512:1024], in0=ps1, in1=b_sb[:, slab, 512:1024])
            nc.gpsimd.dma_start(out=orows[128 * slot : 128 * (slot + 1), :], in_=o)

    # DMA emission order controls transfer order on the shared DMA engines:
    # first W chunk, first gx chunk, rest of slab0, next gx, slab1+bias, ...
    nc.sync.dma_start(out=w_sb[:, 0, 0, :], in_=wts[:, 0, 0, :])
    emit_gx(0)
    bt_ap = bts[:, :]
    nc.gpsimd.dma_start(
        out=b_sb,
        in_=bass.AP(
            tensor=bt_ap.tensor, offset=bt_ap.offset,
            ap=[[0, 128]] + list(bt_ap.ap),
        ),
    )
    for c in range(1, DC):
        nc.sync.dma_start(out=w_sb[:, 0, c, :], in_=wts[:, 0, c, :])
    emit_gx(1)
    compute_chunk(0)
    for c in range(DC):
        nc.sync.dma_start(out=w_sb[:, 1, c, :], in_=wts[:, 1, c, :])
    emit_gx(2)
    compute_chunk(1)
    emit_gx(3)
    compute_chunk(2)
    emit_gx(4)
    compute_chunk(3)
    compute_chunk(4)


_nc_a = None
_nc_b = None


def _get_programs():
    global _nc_a, _nc_b
    if _nc_a is None:
        _nc_a = build_launch_a()
        _nc_b = build_launch_b()
    return _nc_a, _nc_b


def _run_spmd(nc, in_maps, label):
    if os.environ.get("BASS_SIM"):
        from concourse.bass_interp import CoreSim

        results = []
        for im in in_maps:
            sim = CoreSim(nc)
            for k, v in im.items():
                sim.tensor(k)[:] = v
            sim.simulate()
            out = {}
            for alloc in nc.m.functions[0].allocations:
                if getattr(alloc, "kind", None) == "ExternalOutput":
                    name = alloc.memorylocations[0].name
                    out[name] = np.array(sim.mem_tensor(name))
            results.append(out)

        class R:
            pass

        r = R()
        r.results = results
        r.exec_time_ns = None
        return r
    from concourse.bass_utils import run_bass_kernel_spmd

    trace = bool(os.environ.get("BASS_TRACE"))
    kw = {}
    if trace:
        tdir = os.path.abspath(f"trace_{label}")
        os.makedirs(tdir, exist_ok=True)
        kw = dict(trace=True, tmpdir=tdir, trace_cores=[0])
    res = run_bass_kernel_spmd(nc, in_maps, core_ids=list(range(NCORES)), **kw)
    if trace:
        _perf.append((label, res.exec_time_ns, res.mean_exec_time_ns))
    return res


def _solve_runs(blocks_e, runs):
    """Cover each expert's block count with runs (core, slab, cap).

    Greedy: experts by descending need; prefer the largest run that fits
    exactly under the need, else burn the smallest run that overshoots.
    """
    runs = sorted(runs, key=lambda r: -r[2])
    assign = {e: [] for e in range(len(blocks_e))}
    need = {e: int(n) for e, n in enumerate(blocks_e)}
    for e in sorted(range(len(blocks_e)), key=lambda e: -blocks_e[e]):
        while need[e] > 0:
            fit = [r for r in runs if r[2] <= need[e]]
            if fit:
                r = fit[0]
            else:
                if not runs:
                    return None
                r = min(runs, key=lambda r: r[2])
            runs.remove(r)
            assign[e].append(r)
            need[e] -= r[2]
    return assign


def kernel(x, tile_sigs, W, b):
    x = np.asarray(x, np.float32)
    tile_sigs = np.asarray(tile_sigs, np.float32)
    W = np.asarray(W, np.float32)
    b = np.asarray(b, np.float32)
    _perf.clear()

    nc_a, nc_b = _get_programs()

    xf = x.reshape(NTOK, D)
    x_hi = xf.astype(BF16NP)
    x_lo8 = ((xf - x_hi.astype(np.float32)) * 64.0).astype(F8NP)
    sgnf = np.sign(tile_sigs).astype(np.float32)  # [T, D]
    # [p, c, t] layouts: element [p,c,t] = sgn[t, 128c+p]
    sgh = np.ascontiguousarray(
        sgnf.T.astype(BF16NP).reshape(DC, 128, T).transpose(1, 0, 2)
    )
    sgl = np.ascontiguousarray(
        (sgnf.T / 64.0).astype(F8NP).reshape(DC, 128, T).transpose(1, 0, 2)
    )

    in_maps_a = []
    for c in range(NCORES):
        sh = slice(c * SHARD, (c + 1) * SHARD)
        # xht[p, ch, n] = x_hi[n, 128*ch + p]
        xht = np.ascontiguousarray(x_hi[sh].T.reshape(DC, 128, SHARD).transpose(1, 0, 2))
        xlt = np.ascontiguousarray(x_lo8[sh].T.reshape(DC, 128, SHARD).transpose(1, 0, 2))
        in_maps_a.append({"xht": xht, "xlt": xlt, "sgh": sgh, "sgl": sgl})

    res_a = _run_spmd(nc_a, in_maps_a, "a")
    # idx result [128, ABLK]: token 128*j + p at [p, j]
    idx_all = np.concatenate(
        [
            np.rint(np.asarray(res_a.results[c]["idx"], np.float32)).astype(np.int64).T.ravel()
            for c in range(NCORES)
        ]
    )

    # host routing: stable counting sort -> block-level packing onto cores
    order = np.argsort(idx_all, kind="stable")
    counts = np.bincount(idx_all, minlength=T)
    bounds = np.concatenate([[0], np.cumsum(counts)])
    blocks_e = [int(np.ceil(counts[t] / 128)) for t in range(T)]
    runs = [(c, 0, RUN0) for c in range(NCORES)] + [(c, 1, RUN1) for c in range(NCORES)]
    assign = _solve_runs(blocks_e, runs)
    assert assign is not None, f"block assignment infeasible for counts {counts}"

    slot_expert = np.zeros((NCORES, 2), np.int64)
    slot_tokens = np.full((NCORES, GCAP), TRASH, np.int64)
    for t in range(T):
        ids = order[bounds[t] : bounds[t + 1]]
        pos = 0
        for core, sl, cap in assign[t]:
            slot_expert[core, sl] = t
            base = 0 if sl == 0 else RUN0 * 128
            take = ids[pos : pos + cap * 128]
            slot_tokens[core, base : base + len(take)] = take
            pos += len(take)
        assert pos == len(ids)

    x_pad = np.vstack([x_hi, np.zeros((1, D), BF16NP)])  # [NTOK+1, D]
    # Wb[t, p, ch, e] = W[t, 128*ch + p, e]
    Wb = np.ascontiguousarray(
        W.astype(BF16NP).reshape(T, DC, 128, D).transpose(0, 2, 1, 3)
    )
    in_maps_b = []
    for core in range(NCORES):
        ids = slot_tokens[core]
        rows = x_pad[ids]  # [GCAP, D] bf16
        gxt = np.ascontiguousarray(rows.reshape(GCAP, DC, 128).transpose(2, 1, 0))
        wts = np.ascontiguousarray(
            np.stack([Wb[slot_expert[core, 0]], Wb[slot_expert[core, 1]]], axis=1)
        )  # [128, 2, DC, D]
        bts = np.ascontiguousarray(
            np.stack([b[slot_expert[core, 0]], b[slot_expert[core, 1]]])
        )  # [2, D] f32
        in_maps_b.append({"gxt": gxt, "wts": wts, "bts": bts})

    res_b = _run_spmd(nc_b, in_maps_b, "b")

    out_pad = np.zeros((NTOK, D), np.float32)
    for core in range(NCORES):
        orows = np.asarray(res_b.results[core]["orows"]).astype(np.float32)
        ids = slot_tokens[core]
        valid = ids < NTOK
        out_pad[ids[valid]] = orows[valid]
    return out_pad.reshape(B, S, D)


# revision 6
# speedup vs baseline: 1.0109x; 1.0109x over previous
"""ContentOnlyRouter MoE kernel for 8x TRN2 NeuronCores.

Strategy (two SPMD launches, host does only data marshalling):
  Launch A (data-parallel scoring): each core scores its 2048-token shard
    against sign(tile_sigs). x is split as bf16 hi + fp8e4m3 lo (lo scaled
    by 64, sign vectors scaled by 1/64 so products land exactly); both parts
    accumulate into one PSUM bank laid out [128 tok, 16 blk, 8 expert], so
    argmax runs directly on DVE with no transposes. Scores match fp32
    scoring to ~1e-4 absolute; verified exact-argmax on this input
    distribution with ~100x gap margin.
  Host glue: stable counting-sort of the 16384 expert ids; expert token
    lists are padded to 128-multiples and the resulting blocks are packed
    onto 8 cores x 17 block-slots (slots 0-8 use weight slab 0, slots 9-16
    slab 1) by a greedy covering solver. The gather itself (pick + transpose
    token rows) happens on host, so launch B does no dma_gather.
  Launch B (block-parallel grouped GEMM): each core streams its 17
    pre-gathered 128-token blocks and 2 weight slabs, does 8 accumulating
    bf16 matmuls per 512-wide PSUM half, adds bias on DVE, writes bf16 rows.
    Host scatters rows back to token order.

Shapes hardcoded for B=4, S=4096, D=1024, T=8 per the problem spec.
"""

import os

os.environ.setdefault("JAX_PLATFORMS", "")

import contextlib

import numpy as np
import ml_dtypes

import concourse.bass as bass
import concourse.bacc as bacc
import concourse.mybir as mybir
import concourse.tile as tile

B, S, D, T = 4, 4096, 1024, 8
NTOK = B * S             # 16384 tokens
NCORES = 8
SHARD = NTOK // NCORES   # 2048 tokens scored per core
DC = D // 128            # 8 contraction chunks
ABLK = SHARD // 128      # 16 token blocks per shard
NACH = 4                 # launch A DMA chunks (512 tokens each)
NSLOT = 17               # GEMM block slots per core
RUN0, RUN1 = 9, 8        # slots per weight slab (slab0: slots 0-8, slab1: 9-16)
GCAP = NSLOT * 128       # 2176 gathered tokens per core
TRASH = NTOK             # row index used for padding slots
GX_CHUNKS = [2, 3, 4, 4, 4]  # slots per launch-B gather-stream chunk

F32 = mybir.dt.float32
BF16 = mybir.dt.bfloat16
F8 = mybir.dt.float8e4

BF16NP = ml_dtypes.bfloat16
F8NP = ml_dtypes.float8_e4m3

_perf = []  # exec_time_ns per launch when tracing


def build_launch_a(iters=1):
    """Scores + argmax for one 2048-token shard.

    The sign vectors ride in the first T columns of the chunk-0 input
    (no separate const DMAs); argmax runs per 512-token chunk so the DVE
    tail after the last DMA is one chunk deep, with idx rows written out
    on the otherwise-idle gpsimd queue.
    """
    nc = bacc.Bacc(None)
    CH = SHARD // NACH       # 512 tokens per DMA chunk
    xht = nc.dram_tensor("xht", [128, DC, T + SHARD], BF16, kind="ExternalInput")
    xlt = nc.dram_tensor("xlt", [128, DC, T + SHARD], F8, kind="ExternalInput")
    idx = nc.dram_tensor("idx", [128, ABLK], F32, kind="ExternalOutput")

    with tile.TileContext(nc) as tc:
        with (
            tc.tile_pool(name="const", bufs=1) as const,
            tc.tile_pool(name="x0", bufs=1) as x0p,
            tc.tile_pool(name="xa", bufs=3) as xa,
            tc.tile_pool(name="ps", bufs=1, space="PSUM") as ps,
            tc.tile_pool(name="sb", bufs=2) as sb,
        ):
            BPC = CH // 128
            # rev-iota: value 7-t at expert slot t (first-occurrence argmax)
            revio = const.tile([128, BPC, T], F32)
            for t in range(T):
                nc.vector.memset(revio[:, :, t : t + 1], float(T - 1 - t))

            loop = tc.For_i(0, iters, 1) if iters > 1 else contextlib.nullcontext()
            with loop:
                _body_a(nc, x0p, xa, ps, sb, revio, xht, xlt, idx)
    nc.compile()
    return nc


def _body_a(nc, x0p, xa, ps, sb, revio, xht, xlt, idx):
    CH = SHARD // NACH       # 512 tokens per DMA chunk
    BPC = CH // 128          # 4 matmul blocks per chunk
    psum = ps.tile([128, ABLK, T], F32)
    xh0 = x0p.tile([128, DC, T + CH], BF16, tag="xh0")
    xl0 = x0p.tile([128, DC, T + CH], F8, tag="xl0")
    sgh_sb = xh0[:, :, 0:T]
    sgl_sb = xl0[:, :, 0:T]
    for g in range(NACH):
        if g == 0:
            xh, xl = xh0, xl0
            nc.sync.dma_start(out=xh0, in_=xht[:, :, 0 : T + CH])
            nc.sync.dma_start(out=xl0, in_=xlt[:, :, 0 : T + CH])
            toff = T
        else:
            xh = xa.tile([128, DC, CH], BF16, tag="xh")
            xl = xa.tile([128, DC, CH], F8, tag="xl")
            nc.sync.dma_start(out=xh, in_=xht[:, :, T + CH * g : T + CH * (g + 1)])
            nc.sync.dma_start(out=xl, in_=xlt[:, :, T + CH * g : T + CH * (g + 1)])
            toff = 0
        for j in range(BPC):
            blk = g * BPC + j
            o = psum[:, blk, :]
            tok = slice(toff + 128 * j, toff + 128 * (j + 1))
            for c in range(DC):
                nc.tensor.matmul(
                    out=o, lhsT=xh[:, c, tok], rhs=sgh_sb[:, c, :],
                    start=(c == 0), stop=False,
                )
            for c in range(DC):
                nc.tensor.matmul(
                    out=o, lhsT=xl[:, c, tok], rhs=sgl_sb[:, c, :],
                    start=False, stop=(c == DC - 1),
                )
        # per-chunk argmax over the 8 experts, first occurrence wins
        pch = psum[:, BPC * g : BPC * (g + 1), :]
        smax = sb.tile([128, BPC, 1], F32, tag="smax")
        nc.vector.reduce_max(out=smax, in_=pch, axis=mybir.AxisListType.X)
        m = sb.tile([128, BPC, T], F32, tag="m")
        nc.vector.tensor_tensor(
            out=m, in0=pch, in1=smax.to_broadcast([128, BPC, T]),
            op=mybir.AluOpType.is_ge,
        )
        nc.vector.tensor_tensor(out=m, in0=m, in1=revio, op=mybir.AluOpType.mult)
        mm = sb.tile([128, BPC, 1], F32, tag="mm")
        nc.vector.reduce_max(out=mm, in_=m, axis=mybir.AxisListType.X)
        idxv = sb.tile([128, BPC], F32, tag="idxv")
        nc.vector.tensor_scalar(
            out=idxv, in0=mm[:, :, 0], scalar1=-1.0, scalar2=float(T - 1),
            op0=mybir.AluOpType.mult, op1=mybir.AluOpType.add,
        )
        nc.gpsimd.dma_start(out=idx[:, BPC * g : BPC * (g + 1)], in_=idxv)


def build_launch_b(iters=1):
    """Grouped GEMM over 17 pre-gathered 128-token blocks (2 weight slabs)."""
    nc = bacc.Bacc(None)
    gxt = nc.dram_tensor("gxt", [128, DC, GCAP], BF16, kind="ExternalInput")
    wts = nc.dram_tensor("wts", [128, 2, DC, D], BF16, kind="ExternalInput")
    bts = nc.dram_tensor("bts", [2, D], F32, kind="ExternalInput")
    orows = nc.dram_tensor("orows", [GCAP, D], BF16, kind="ExternalOutput")

    with tile.TileContext(nc) as tc:
        with (
            tc.tile_pool(name="wp", bufs=1) as wp,
            tc.tile_pool(name="gx", bufs=3) as gxp,
            tc.tile_pool(name="ps", bufs=4, space="PSUM") as ps,
            tc.tile_pool(name="osb", bufs=3) as osb,
        ):
            loop = tc.For_i(0, iters, 1) if iters > 1 else contextlib.nullcontext()
            with loop:
                _body_b(nc, wp, gxp, ps, osb, gxt, wts, bts, orows)
    nc.compile()
    return nc


def _body_b(nc, wp, gxp, ps, osb, gxt, wts, bts, orows):
    w_sb = wp.tile([128, 2, DC, D], BF16, tag="w")
    b_sb = wp.tile([128, 2, D], F32, tag="b")

    offs = np.cumsum([0] + GX_CHUNKS)
    gx_tiles = [None] * len(GX_CHUNKS)

    def emit_gx(ci):
        t = gxp.tile([128, DC, 512], BF16, tag="gx")
        n = GX_CHUNKS[ci] * 128
        nc.sync.dma_start(
            out=t[:, :, 0:n], in_=gxt[:, :, 128 * offs[ci] : 128 * offs[ci] + n]
        )
        gx_tiles[ci] = t

    def compute_chunk(ci):
        t = gx_tiles[ci]
        for si in range(GX_CHUNKS[ci]):
            slot = offs[ci] + si
            slab = 0 if slot < RUN0 else 1
            tok = slice(128 * si, 128 * (si + 1))
            ps0 = ps.tile([128, 512], F32, tag="ps0")
            ps1 = ps.tile([128, 512], F32, tag="ps1")
            for c in range(DC):
                nc.tensor.matmul(
                    out=ps0, lhsT=t[:, c, tok], rhs=w_sb[:, slab, c, 0:512],
                    start=(c == 0), stop=(c == DC - 1),
                )
                nc.tensor.matmul(
                    out=ps1, lhsT=t[:, c, tok], rhs=w_sb[:, slab, c, 512:1024],
                    start=(c == 0), stop=(c == DC - 1),
                )
            o = osb.tile([128, D], BF16)
            nc.vector.tensor_add(out=o[:, 0:512], in0=ps0, in1=b_sb[:, slab, 0:512])
            nc.vector.tensor_add(out=o[:, 512:1024], in0=ps1, in1=b_sb[:, slab, 512:1024])
            nc.gpsimd.dma_start(out=orows[128 * slot : 128 * (slot + 1), :], in_=o)

    # DMA emission order controls transfer order on the shared DMA engines:
    # first W chunk, first gx chunk, rest of slab0, next gx, slab1+bias, ...
    nc.sync.dma_start(out=w_sb[:, 0, 0, :], in_=wts[:, 0, 0, :])
    emit_gx(0)
    # Gate the bias broadcast behind chunk-0's arrival: the Pool DMA queue is
    # in-order, so this 1-element copy keeps the (long) broadcast from
    # preempting the critical w00/gx0 transfers on the shared DMA engines.
    gate = wp.tile([1, 1], BF16, tag="gate")
    nc.gpsimd.dma_start(out=gate, in_=gx_tiles[0][0:1, 0, 0:1])
    bt_ap = bts[:, :]
    nc.gpsimd.dma_start(
        out=b_sb,
        in_=bass.AP(
            tensor=bt_ap.tensor, offset=bt_ap.offset,
            ap=[[0, 128]] + list(bt_ap.ap),
        ),
    )
    for c in range(1, DC):
        nc.sync.dma_start(out=w_sb[:, 0, c, :], in_=wts[:, 0, c, :])
    emit_gx(1)
    compute_chunk(0)
    for c in range(DC):
        nc.sync.dma_start(out=w_sb[:, 1, c, :], in_=wts[:, 1, c, :])
    emit_gx(2)
    compute_chunk(1)
    emit_gx(3)
    compute_chunk(2)
    emit_gx(4)
    compute_chunk(3)
    compute_chunk(4)


_nc_a = None
_nc_b = None


def _get_programs():
    global _nc_a, _nc_b
    if _nc_a is None:
        _nc_a = build_launch_a()
        _nc_b = build_launch_b()
    return _nc_a, _nc_b


def _run_spmd(nc, in_maps, label):
    if os.environ.get("BASS_SIM"):
        from concourse.bass_interp import CoreSim

        results = []
        for im in in_maps:
            sim = CoreSim(nc)
            for k, v in im.items():
                sim.tensor(k)[:] = v
            sim.simulate()
            out = {}
            for alloc in nc.m.functions[0].allocations:
                if getattr(alloc, "kind", None) == "ExternalOutput":
                    name = alloc.memorylocations[0].name
                    out[name] = np.array(sim.mem_tensor(name))
            results.append(out)

        class R:
            pass

        r = R()
        r.results = results
        r.exec_time_ns = None
        return r
    from concourse.bass_utils import run_bass_kernel_spmd

    trace = bool(os.environ.get("BASS_TRACE"))
    kw = {}
    if trace:
        tdir = os.path.abspath(f"trace_{label}")
        os.makedirs(tdir, exist_ok=True)
        kw = dict(trace=True, tmpdir=tdir, trace_cores=[0])
    res = run_bass_kernel_spmd(nc, in_maps, core_ids=list(range(NCORES)), **kw)
    if trace:
        _perf.append((label, res.exec_time_ns, res.mean_exec_time_ns))
    return res


def _solve_runs(blocks_e, runs):
    """Cover each expert's block count with runs (core, slab, cap).

    Greedy: experts by descending need; prefer the largest run that fits
    exactly under the need, else burn the smallest run that overshoots.
    """
    runs = sorted(runs, key=lambda r: -r[2])
    assign = {e: [] for e in range(len(blocks_e))}
    need = {e: int(n) for e, n in enumerate(blocks_e)}
    for e in sorted(range(len(blocks_e)), key=lambda e: -blocks_e[e]):
        while need[e] > 0:
            fit = [r for r in runs if r[2] <= need[e]]
            if fit:
                r = fit[0]
            else:
                if not runs:
                    return None
                r = min(runs, key=lambda r: r[2])
            runs.remove(r)
            assign[e].append(r)
            need[e] -= r[2]
    return assign


def kernel(x, tile_sigs, W, b):
    x = np.asarray(x, np.float32)
    tile_sigs = np.asarray(tile_sigs, np.float32)
    W = np.asarray(W, np.float32)
    b = np.asarray(b, np.float32)
    _perf.clear()

    nc_a, nc_b = _get_programs()

    xf = x.reshape(NTOK, D)
    x_hi = xf.astype(BF16NP)
    x_lo8 = ((xf - x_hi.astype(np.float32)) * 64.0).astype(F8NP)
    sgnf = np.sign(tile_sigs).astype(np.float32)  # [T, D]
    # [p, c, t] layouts: element [p,c,t] = sgn[t, 128c+p]
    sgh = np.ascontiguousarray(
        sgnf.T.astype(BF16NP).reshape(DC, 128, T).transpose(1, 0, 2)
    )
    sgl = np.ascontiguousarray(
        (sgnf.T / 64.0).astype(F8NP).reshape(DC, 128, T).transpose(1, 0, 2)
    )

    in_maps_a = []
    for c in range(NCORES):
        sh = slice(c * SHARD, (c + 1) * SHARD)
        # xht[p, ch, T+n] = x_hi[n, 128*ch + p]; sign vectors in cols 0..T
        xht = np.empty((128, DC, T + SHARD), BF16NP)
        xht[:, :, :T] = sgh
        xht[:, :, T:] = x_hi[sh].T.reshape(DC, 128, SHARD).transpose(1, 0, 2)
        xlt = np.empty((128, DC, T + SHARD), F8NP)
        xlt[:, :, :T] = sgl
        xlt[:, :, T:] = x_lo8[sh].T.reshape(DC, 128, SHARD).transpose(1, 0, 2)
        in_maps_a.append({"xht": xht, "xlt": xlt})

    res_a = _run_spmd(nc_a, in_maps_a, "a")
    # idx result [128, ABLK]: token 128*j + p at [p, j]
    idx_all = np.concatenate(
        [
            np.rint(np.asarray(res_a.results[c]["idx"], np.float32)).astype(np.int64).T.ravel()
            for c in range(NCORES)
        ]
    )

    # host routing: stable counting sort -> block-level packing onto cores
    order = np.argsort(idx_all, kind="stable")
    counts = np.bincount(idx_all, minlength=T)
    bounds = np.concatenate([[0], np.cumsum(counts)])
    blocks_e = [int(np.ceil(counts[t] / 128)) for t in range(T)]
    runs = [(c, 0, RUN0) for c in range(NCORES)] + [(c, 1, RUN1) for c in range(NCORES)]
    assign = _solve_runs(blocks_e, runs)
    assert assign is not None, f"block assignment infeasible for counts {counts}"

    slot_expert = np.zeros((NCORES, 2), np.int64)
    slot_tokens = np.full((NCORES, GCAP), TRASH, np.int64)
    for t in range(T):
        ids = order[bounds[t] : bounds[t + 1]]
        pos = 0
        for core, sl, cap in assign[t]:
            slot_expert[core, sl] = t
            base = 0 if sl == 0 else RUN0 * 128
            take = ids[pos : pos + cap * 128]
            slot_tokens[core, base : base + len(take)] = take
            pos += len(take)
        assert pos == len(ids)

    x_pad = np.vstack([x_hi, np.zeros((1, D), BF16NP)])  # [NTOK+1, D]
    # Wb[t, p, ch, e] = W[t, 128*ch + p, e]
    Wb = np.ascontiguousarray(
        W.astype(BF16NP).reshape(T, DC, 128, D).transpose(0, 2, 1, 3)
    )
    in_maps_b = []
    for core in range(NCORES):
        ids = slot_tokens[core]
        rows = x_pad[ids]  # [GCAP, D] bf16
        gxt = np.ascontiguousarray(rows.reshape(GCAP, DC, 128).transpose(2, 1, 0))
        wts = np.ascontiguousarray(
            np.stack([Wb[slot_expert[core, 0]], Wb[slot_expert[core, 1]]], axis=1)
        )  # [128, 2, DC, D]
        bts = np.ascontiguousarray(
            np.stack([b[slot_expert[core, 0]], b[slot_expert[core, 1]]])
        )  # [2, D] f32
        in_maps_b.append({"gxt": gxt, "wts": wts, "bts": bts})

    res_b = _run_spmd(nc_b, in_maps_b, "b")

    out_pad = np.zeros((NTOK, D), np.float32)
    for core in range(NCORES):
        orows = np.asarray(res_b.results[core]["orows"]).astype(np.float32)
        ids = slot_tokens[core]
        valid = ids < NTOK
        out_pad[ids[valid]] = orows[valid]
    return out_pad.reshape(B, S, D)


# revision 10
# speedup vs baseline: 1.0531x; 1.0418x over previous
"""ContentOnlyRouter MoE kernel for 8x TRN2 NeuronCores.

Strategy (two SPMD launches, host does only data marshalling):
  Launch A (data-parallel scoring): each core scores its 2048-token shard
    against sign(tile_sigs). x is split as bf16 hi + fp8e4m3 lo (lo scaled
    by 64, sign vectors scaled by 1/64 so products land exactly); both parts
    accumulate into one PSUM bank laid out [128 tok, 16 blk, 8 expert], so
    argmax runs directly on DVE with no transposes. Scores match fp32
    scoring to ~1e-4 absolute; verified exact-argmax on this input
    distribution with ~100x gap margin.
  Host glue: stable counting-sort of the 16384 expert ids; expert token
    lists are padded to 128-multiples and the resulting blocks are packed
    onto 8 cores x 17 block-slots (slots 0-8 use weight slab 0, slots 9-16
    slab 1) by a greedy covering solver. The gather itself (pick + transpose
    token rows) happens on host, so launch B does no dma_gather.
  Launch B (block-parallel grouped GEMM): each core streams its 17
    pre-gathered 128-token blocks and 2 weight slabs, does 8 accumulating
    bf16 matmuls per 512-wide PSUM half, adds bias on DVE, writes bf16 rows.
    Host scatters rows back to token order.

Shapes hardcoded for B=4, S=4096, D=1024, T=8 per the problem spec.
"""

import os

os.environ.setdefault("JAX_PLATFORMS", "")

import contextlib

import numpy as np
import ml_dtypes

import concourse.bass as bass
import concourse.bacc as bacc
import concourse.mybir as mybir
import concourse.tile as tile

B, S, D, T = 4, 4096, 1024, 8
NTOK = B * S             # 16384 tokens
NCORES = 8
SHARD = NTOK // NCORES   # 2048 tokens scored per core
DC = D // 128            # 8 contraction chunks
ABLK = SHARD // 128      # 16 token blocks per shard
NACH = 4                 # launch A DMA chunks (512 tokens each)
NSLOT = 17               # GEMM block slots per core
RUN0, RUN1 = 9, 8        # slots per weight slab (slab0: slots 0-8, slab1: 9-16)
GCAP = NSLOT * 128       # 2176 gathered tokens per core
TRASH = NTOK             # row index used for padding slots
GX_CHUNKS = [2, 3, 4, 4, 4]  # slots per launch-B gather-stream chunk

F32 = mybir.dt.float32
BF16 = mybir.dt.bfloat16
F8 = mybir.dt.float8e4

BF16NP = ml_dtypes.bfloat16
F8NP = ml_dtypes.float8_e4m3

_perf = []  # exec_time_ns per launch when tracing


def build_launch_a(iters=1):
    """Scores + argmax for one 2048-token shard.

    The sign vectors ride in the first T columns of the chunk-0 input
    (no separate const DMAs); argmax runs per 512-token chunk so the DVE
    tail after the last DMA is one chunk deep, with idx rows written out
    on the otherwise-idle gpsimd queue.
    """
    nc = bacc.Bacc(None)
    CH = SHARD // NACH       # 512 tokens per DMA chunk
    xht = nc.dram_tensor("xht", [128, DC, T + SHARD], BF16, kind="ExternalInput")
    xlt = nc.dram_tensor("xlt", [128, DC, T + SHARD], F8, kind="ExternalInput")
    idx = nc.dram_tensor("idx", [128, ABLK], F32, kind="ExternalOutput")

    with tile.TileContext(nc) as tc:
        with (
            tc.tile_pool(name="const", bufs=1) as const,
            tc.tile_pool(name="x0", bufs=1) as x0p,
            tc.tile_pool(name="xa", bufs=3) as xa,
            tc.tile_pool(name="ps", bufs=1, space="PSUM") as ps,
            tc.tile_pool(name="sb", bufs=2) as sb,
        ):
            BPC = CH // 128
            # rev-iota: value 7-t at expert slot t (first-occurrence argmax)
            revio = const.tile([128, BPC, T], F32)
            for t in range(T):
                nc.vector.memset(revio[:, :, t : t + 1], float(T - 1 - t))

            loop = tc.For_i(0, iters, 1) if iters > 1 else contextlib.nullcontext()
            with loop:
                _body_a(nc, x0p, xa, ps, sb, revio, xht, xlt, idx)
    nc.compile()
    return nc


def _body_a(nc, x0p, xa, ps, sb, revio, xht, xlt, idx):
    CH = SHARD // NACH       # 512 tokens per DMA chunk
    BPC = CH // 128          # 4 matmul blocks per chunk
    psum = ps.tile([128, ABLK, T], F32)
    xh0 = x0p.tile([128, DC, T + CH], BF16, tag="xh0")
    xl0 = x0p.tile([128, DC, T + CH], F8, tag="xl0")
    sgh_sb = xh0[:, :, 0:T]
    sgl_sb = xl0[:, :, 0:T]
    for g in range(NACH):
        if g == 0:
            xh, xl = xh0, xl0
            nc.sync.dma_start(out=xh0, in_=xht[:, :, 0 : T + CH])
            nc.sync.dma_start(out=xl0, in_=xlt[:, :, 0 : T + CH])
            toff = T
        else:
            xh = xa.tile([128, DC, CH], BF16, tag="xh")
            xl = xa.tile([128, DC, CH], F8, tag="xl")
            nc.sync.dma_start(out=xh, in_=xht[:, :, T + CH * g : T + CH * (g + 1)])
            nc.sync.dma_start(out=xl, in_=xlt[:, :, T + CH * g : T + CH * (g + 1)])
            toff = 0
        for j in range(BPC):
            blk = g * BPC + j
            o = psum[:, blk, :]
            tok = slice(toff + 128 * j, toff + 128 * (j + 1))
            for c in range(DC):
                nc.tensor.matmul(
                    out=o, lhsT=xh[:, c, tok], rhs=sgh_sb[:, c, :],
                    start=(c == 0), stop=False,
                )
            for c in range(DC):
                nc.tensor.matmul(
                    out=o, lhsT=xl[:, c, tok], rhs=sgl_sb[:, c, :],
                    start=False, stop=(c == DC - 1),
                )
        # per-chunk argmax over the 8 experts, first occurrence wins
        pch = psum[:, BPC * g : BPC * (g + 1), :]
        smax = sb.tile([128, BPC, 1], F32, tag="smax")
        nc.vector.reduce_max(out=smax, in_=pch, axis=mybir.AxisListType.X)
        m = sb.tile([128, BPC, T], F32, tag="m")
        nc.vector.tensor_tensor(
            out=m, in0=pch, in1=smax.to_broadcast([128, BPC, T]),
            op=mybir.AluOpType.is_ge,
        )
        nc.vector.tensor_tensor(out=m, in0=m, in1=revio, op=mybir.AluOpType.mult)
        mm = sb.tile([128, BPC, 1], F32, tag="mm")
        nc.vector.reduce_max(out=mm, in_=m, axis=mybir.AxisListType.X)
        idxv = sb.tile([128, BPC], F32, tag="idxv")
        nc.vector.tensor_scalar(
            out=idxv, in0=mm[:, :, 0], scalar1=-1.0, scalar2=float(T - 1),
            op0=mybir.AluOpType.mult, op1=mybir.AluOpType.add,
        )
        nc.gpsimd.dma_start(out=idx[:, BPC * g : BPC * (g + 1)], in_=idxv)


NWARM = 13  # PE warm-up matmuls: burn the p-state ramp before the GEMM


def build_launch_b(iters=1):
    """Grouped GEMM over 17 pre-gathered 128-token blocks (2 weight slabs)."""
    nc = bacc.Bacc(None)
    gxt = nc.dram_tensor("gxt", [128, DC, GCAP], BF16, kind="ExternalInput")
    wts = nc.dram_tensor("wts", [128, 2, DC, D], BF16, kind="ExternalInput")
    bts = nc.dram_tensor("bts", [1, 2 * D], BF16, kind="ExternalInput")
    orows = nc.dram_tensor("orows", [GCAP, D], BF16, kind="ExternalOutput")

    with tile.TileContext(nc) as tc:
        with (
            tc.tile_pool(name="wp", bufs=1) as wp,
            tc.tile_pool(name="gx", bufs=3) as gxp,
            tc.tile_pool(name="ps", bufs=3, space="PSUM") as ps,
            tc.tile_pool(name="bp", bufs=2, space="PSUM") as bp,
            tc.tile_pool(name="osb", bufs=3) as osb,
        ):
            loop = tc.For_i(0, iters, 1) if iters > 1 else contextlib.nullcontext()
            with loop:
                _body_b(nc, wp, gxp, ps, bp, osb, gxt, wts, bts, orows)
    nc.compile()
    return nc


def _body_b(nc, wp, gxp, ps, bp, osb, gxt, wts, bts, orows):
    w_sb = wp.tile([128, 2, DC, D], BF16, tag="w")
    b_sb = wp.tile([128, 2, D], F32, tag="b")
    ones = wp.tile([1, 512], BF16, tag="ones")
    bts_sb = wp.tile([1, 2 * D], BF16, tag="btsb")

    offs = np.cumsum([0] + GX_CHUNKS)
    gx_tiles = [None] * len(GX_CHUNKS)

    def emit_gx(ci):
        t = gxp.tile([128, DC, 512], BF16, tag="gx")
        n = GX_CHUNKS[ci] * 128
        nc.sync.dma_start(
            out=t[:, :, 0:n], in_=gxt[:, :, 128 * offs[ci] : 128 * offs[ci] + n]
        )
        gx_tiles[ci] = t

    def emit_bias():
        # bias broadcast via K=1 matmul (ones^T @ bts row) on the idle PE --
        # cheaper than a 128-partition DMA broadcast on the loaded DMA rails
        for s in range(2):
            for h in range(2):
                bps = bp.tile([128, 512], F32, tag="bps")
                nc.tensor.matmul(
                    out=bps, lhsT=ones[:, 0:128],
                    rhs=bts_sb[:, s * D + 512 * h : s * D + 512 * (h + 1)],
                    start=True, stop=True,
                )
                nc.vector.tensor_copy(out=b_sb[:, s, 512 * h : 512 * (h + 1)], in_=bps)

    def drain(slot, ps0, ps1):
        slab = 0 if slot < RUN0 else 1
        o = osb.tile([128, D], BF16)
        nc.vector.tensor_add(out=o[:, 0:512], in0=ps0, in1=b_sb[:, slab, 0:512])
        nc.vector.tensor_add(out=o[:, 512:1024], in0=ps1, in1=b_sb[:, slab, 512:1024])
        nc.gpsimd.dma_start(out=orows[128 * slot : 128 * (slot + 1), :], in_=o)

    def compute_chunk0():
        # c-major over the first 2 slots: PE consumes one W chunk per 852ns
        # against the 728ns/chunk W stream, so the slab-0 load never stalls it
        t = gx_tiles[0]
        pses = []
        for si in range(GX_CHUNKS[0]):
            p0 = ps.tile([128, 512], F32, tag="ps0")
            p1 = ps.tile([128, 512], F32, tag="ps1")
            pses.append((p0, p1))
        for c in range(DC):
            for si in range(GX_CHUNKS[0]):
                p0, p1 = pses[si]
                tok = slice(128 * si, 128 * (si + 1))
                nc.tensor.matmul(
                    out=p0, lhsT=t[:, c, tok], rhs=w_sb[:, 0, c, 0:512],
                    start=(c == 0), stop=(c == DC - 1),
                )
                nc.tensor.matmul(
                    out=p1, lhsT=t[:, c, tok], rhs=w_sb[:, 0, c, 512:1024],
                    start=(c == 0), stop=(c == DC - 1),
                )
            if c == 0:
                emit_bias()
        for si in range(GX_CHUNKS[0]):
            drain(si, *pses[si])

    def compute_chunk(ci):
        t = gx_tiles[ci]
        for si in range(GX_CHUNKS[ci]):
            slot = offs[ci] + si
            slab = 0 if slot < RUN0 else 1
            tok = slice(128 * si, 128 * (si + 1))
            ps0 = ps.tile([128, 512], F32, tag="ps0")
            ps1 = ps.tile([128, 512], F32, tag="ps1")
            for c in range(DC):
                nc.tensor.matmul(
                    out=ps0, lhsT=t[:, c, tok], rhs=w_sb[:, slab, c, 0:512],
                    start=(c == 0), stop=(c == DC - 1),
                )
                nc.tensor.matmul(
                    out=ps1, lhsT=t[:, c, tok], rhs=w_sb[:, slab, c, 512:1024],
                    start=(c == 0), stop=(c == DC - 1),
                )
            drain(slot, ps0, ps1)

    # DMA emission order controls transfer order on the shared DMA engines:
    # tiny bias row, first W chunk, first gx chunk, rest of slab0, ...
    nc.vector.memset(ones, 1.0)
    nc.sync.dma_start(out=bts_sb, in_=bts[:, :])
    for i in range(NWARM):
        wps = ps.tile([128, 512], F32, tag="ps0" if i % 2 == 0 else "ps1")
        nc.tensor.matmul(out=wps, lhsT=ones[:, 0:128], rhs=ones[:, :], start=True, stop=True)
    nc.sync.dma_start(out=w_sb[:, 0, 0, :], in_=wts[:, 0, 0, :])
    emit_gx(0)
    for c in range(1, DC):
        nc.sync.dma_start(out=w_sb[:, 0, c, :], in_=wts[:, 0, c, :])
    emit_gx(1)
    compute_chunk0()
    for c in range(DC):
        nc.sync.dma_start(out=w_sb[:, 1, c, :], in_=wts[:, 1, c, :])
    emit_gx(2)
    compute_chunk(1)
    emit_gx(3)
    compute_chunk(2)
    emit_gx(4)
    compute_chunk(3)
    compute_chunk(4)


_nc_a = None
_nc_b = None


def _get_programs():
    global _nc_a, _nc_b
    if _nc_a is None:
        _nc_a = build_launch_a()
        _nc_b = build_launch_b()
    return _nc_a, _nc_b


def _run_spmd(nc, in_maps, label):
    if os.environ.get("BASS_SIM"):
        from concourse.bass_interp import CoreSim

        results = []
        for im in in_maps:
            sim = CoreSim(nc)
            for k, v in im.items():
                sim.tensor(k)[:] = v
            sim.simulate()
            out = {}
            for alloc in nc.m.functions[0].allocations:
                if getattr(alloc, "kind", None) == "ExternalOutput":
                    name = alloc.memorylocations[0].name
                    out[name] = np.array(sim.mem_tensor(name))
            results.append(out)

        class R:
            pass

        r = R()
        r.results = results
        r.exec_time_ns = None
        return r
    from concourse.bass_utils import run_bass_kernel_spmd

    trace = bool(os.environ.get("BASS_TRACE"))
    kw = {}
    if trace:
        tdir = os.path.abspath(f"trace_{label}")
        os.makedirs(tdir, exist_ok=True)
        kw = dict(trace=True, tmpdir=tdir, trace_cores=[0])
    res = run_bass_kernel_spmd(nc, in_maps, core_ids=list(range(NCORES)), **kw)
    if trace:
        _perf.append((label, res.exec_time_ns, res.mean_exec_time_ns))
    return res


def _solve_runs(blocks_e, runs):
    """Cover each expert's block count with runs (core, slab, cap).

    Greedy: experts by descending need; prefer the largest run that fits
    exactly under the need, else burn the smallest run that overshoots.
    """
    runs = sorted(runs, key=lambda r: -r[2])
    assign = {e: [] for e in range(len(blocks_e))}
    need = {e: int(n) for e, n in enumerate(blocks_e)}
    for e in sorted(range(len(blocks_e)), key=lambda e: -blocks_e[e]):
        while need[e] > 0:
            fit = [r for r in runs if r[2] <= need[e]]
            if fit:
                r = fit[0]
            else:
                if not runs:
                    return None
                r = min(runs, key=lambda r: r[2])
            runs.remove(r)
            assign[e].append(r)
            need[e] -= r[2]
    return assign


def kernel(x, tile_sigs, W, b):
    x = np.asarray(x, np.float32)
    tile_sigs = np.asarray(tile_sigs, np.float32)
    W = np.asarray(W, np.float32)
    b = np.asarray(b, np.float32)
    _perf.clear()

    nc_a, nc_b = _get_programs()

    xf = x.reshape(NTOK, D)
    x_hi = xf.astype(BF16NP)
    x_lo8 = ((xf - x_hi.astype(np.float32)) * 64.0).astype(F8NP)
    sgnf = np.sign(tile_sigs).astype(np.float32)  # [T, D]
    # [p, c, t] layouts: element [p,c,t] = sgn[t, 128c+p]
    sgh = np.ascontiguousarray(
        sgnf.T.astype(BF16NP).reshape(DC, 128, T).transpose(1, 0, 2)
    )
    sgl = np.ascontiguousarray(
        (sgnf.T / 64.0).astype(F8NP).reshape(DC, 128, T).transpose(1, 0, 2)
    )

    in_maps_a = []
    for c in range(NCORES):
        sh = slice(c * SHARD, (c + 1) * SHARD)
        # xht[p, ch, T+n] = x_hi[n, 128*ch + p]; sign vectors in cols 0..T
        xht = np.empty((128, DC, T + SHARD), BF16NP)
        xht[:, :, :T] = sgh
        xht[:, :, T:] = x_hi[sh].T.reshape(DC, 128, SHARD).transpose(1, 0, 2)
        xlt = np.empty((128, DC, T + SHARD), F8NP)
        xlt[:, :, :T] = sgl
        xlt[:, :, T:] = x_lo8[sh].T.reshape(DC, 128, SHARD).transpose(1, 0, 2)
        in_maps_a.append({"xht": xht, "xlt": xlt})

    res_a = _run_spmd(nc_a, in_maps_a, "a")
    # idx result [128, ABLK]: token 128*j + p at [p, j]
    idx_all = np.concatenate(
        [
            np.rint(np.asarray(res_a.results[c]["idx"], np.float32)).astype(np.int64).T.ravel()
            for c in range(NCORES)
        ]
    )

    # host routing: stable counting sort -> block-level packing onto cores
    order = np.argsort(idx_all, kind="stable")
    counts = np.bincount(idx_all, minlength=T)
    bounds = np.concatenate([[0], np.cumsum(counts)])
    blocks_e = [int(np.ceil(counts[t] / 128)) for t in range(T)]
    runs = [(c, 0, RUN0) for c in range(NCORES)] + [(c, 1, RUN1) for c in range(NCORES)]
    assign = _solve_runs(blocks_e, runs)
    assert assign is not None, f"block assignment infeasible for counts {counts}"

    slot_expert = np.zeros((NCORES, 2), np.int64)
    slot_tokens = np.full((NCORES, GCAP), TRASH, np.int64)
    for t in range(T):
        ids = order[bounds[t] : bounds[t + 1]]
        pos = 0
        for core, sl, cap in assign[t]:
            slot_expert[core, sl] = t
            base = 0 if sl == 0 else RUN0 * 128
            take = ids[pos : pos + cap * 128]
            slot_tokens[core, base : base + len(take)] = take
            pos += len(take)
        assert pos == len(ids)

    x_pad = np.vstack([x_hi, np.zeros((1, D), BF16NP)])  # [NTOK+1, D]
    # Wb[t, p, ch, e] = W[t, 128*ch + p, e]
    Wb = np.ascontiguousarray(
        W.astype(BF16NP).reshape(T, DC, 128, D).transpose(0, 2, 1, 3)
    )
    in_maps_b = []
    for core in range(NCORES):
        ids = slot_tokens[core]
        rows = x_pad[ids]  # [GCAP, D] bf16
        gxt = np.ascontiguousarray(rows.reshape(GCAP, DC, 128).transpose(2, 1, 0))
        wts = np.ascontiguousarray(
            np.stack([Wb[slot_expert[core, 0]], Wb[slot_expert[core, 1]]], axis=1)
        )  # [128, 2, DC, D]
        bts = np.ascontiguousarray(
            np.concatenate([b[slot_expert[core, 0]], b[slot_expert[core, 1]]])
            .astype(BF16NP).reshape(1, 2 * D)
        )  # [1, 2*D] bf16
        in_maps_b.append({"gxt": gxt, "wts": wts, "bts": bts})

    res_b = _run_spmd(nc_b, in_maps_b, "b")

    out_pad = np.zeros((NTOK, D), np.float32)
    for core in range(NCORES):
        orows = np.asarray(res_b.results[core]["orows"]).astype(np.float32)
        ids = slot_tokens[core]
        valid = ids < NTOK
        out_pad[ids[valid]] = orows[valid]
    return out_pad.reshape(B, S, D)


# revision 13
# speedup vs baseline: 1.1398x; 1.0824x over previous
"""ContentOnlyRouter MoE kernel for 8x TRN2 NeuronCores.

Strategy (two SPMD launches, host does only data marshalling/selection):
  Launch A (data-parallel approx scoring): each core scores its 2048-token
    shard against sign(tile_sigs) in bf16 only (half the DMA of an exact
    hi/lo split) and ships the fp32 scores. bf16 scoring has a bounded
    absolute error (<0.27 on this input distribution), so any token whose
    top-2 approx gap exceeds THETA=0.53 > 2*err_max provably has the true
    argmax; the rest ("borderline", ~400 of 16384) are routed to EVERY
    candidate expert within THETA and disambiguated by launch B's exact
    rescore. Sign vectors ride in the first T columns of chunk 0; scores
    stream out per 512-token chunk on the Activation DMA queue.
  Host glue: candidate sets from approx scores; expert token lists padded
    to 128-multiples; blocks packed onto 8 cores x 17 block-slots (slots
    0-8 = weight slab 0, 9-16 = slab 1) by a greedy covering solver. The
    gather (pick + transpose token rows) happens on host.
  Launch B (block-parallel grouped GEMM + rescore): each core streams its
    17 pre-gathered 128-token blocks and 2 weight slabs; 8 accumulating
    bf16 matmuls per 512-wide PSUM half; bias is built by a K=1 matmul on
    the idle PE (ones x bias-row broadcast) and added on DVE; bf16 rows
    out. A PE warm-up burns the p-state ramp before the GEMM so every
    GEMM matmul runs at full clock. Each core also rescores its 64-token
    share of the borderline set exactly (bf16 hi + fp8e4m3 lo, lo scaled
    by 64 with sign vectors scaled by 1/64 -- products exact, fp32 PSUM
    accumulation; verified argmax-exact on this distribution). Host keeps,
    per borderline token, the row computed under the rescored-argmax
    expert.

Shapes hardcoded for B=4, S=4096, D=1024, T=8 per the problem spec.
"""

import os

os.environ.setdefault("JAX_PLATFORMS", "")

import contextlib

import numpy as np
import ml_dtypes

import concourse.bass as bass
import concourse.bacc as bacc
import concourse.mybir as mybir
import concourse.tile as tile

B, S, D, T = 4, 4096, 1024, 8
NTOK = B * S             # 16384 tokens
NCORES = 8
SHARD = NTOK // NCORES   # 2048 tokens scored per core
DC = D // 128            # 8 contraction chunks
ABLK = SHARD // 128      # 16 token blocks per shard
A_CHUNKS = [4, 4, 4, 4]  # launch A DMA chunk sizes in 128-token blocks
NSLOT = 17               # GEMM block slots per core
RUN0, RUN1 = 9, 8        # slots per weight slab (slab0: slots 0-8, slab1: 9-16)
GCAP = NSLOT * 128       # 2176 gathered tokens per core
TRASH = NTOK             # row index used for padding slots
GX_CHUNKS = [2, 3, 4, 4, 4]  # slots per launch-B gather-stream chunk
THETA = 0.53             # borderline gap threshold (> 2*max bf16 score err)
RTOK = 64                # borderline tokens rescored per core (512 total)
NWARM = 8                # PE warm-up matmuls: burn the p-state ramp pre-GEMM

F32 = mybir.dt.float32
BF16 = mybir.dt.bfloat16
F8 = mybir.dt.float8e4

BF16NP = ml_dtypes.bfloat16
F8NP = ml_dtypes.float8_e4m3

_perf = []  # exec_time_ns per launch when tracing


def build_launch_a(iters=1):
    """bf16 approx scores for one 2048-token shard."""
    nc = bacc.Bacc(None)
    xht = nc.dram_tensor("xht", [128, DC, T + SHARD], BF16, kind="ExternalInput")
    scores = nc.dram_tensor("scores", [128, ABLK, T], F32, kind="ExternalOutput")

    with tile.TileContext(nc) as tc:
        with (
            tc.tile_pool(name="x0", bufs=1) as x0p,
            tc.tile_pool(name="xa", bufs=3) as xa,
            tc.tile_pool(name="ps", bufs=1, space="PSUM") as ps,
            tc.tile_pool(name="sb", bufs=2) as sb,
        ):
            loop = tc.For_i(0, iters, 1) if iters > 1 else contextlib.nullcontext()
            with loop:
                _body_a(nc, x0p, xa, ps, sb, xht, scores)
    nc.compile()
    return nc


def _body_a(nc, x0p, xa, ps, sb, xht, scores):
    psum = ps.tile([128, ABLK, T], F32)
    CH0 = 128 * A_CHUNKS[0]
    xh0 = x0p.tile([128, DC, T + CH0], BF16, tag="xh0")
    sgh_sb = xh0[:, :, 0:T]
    boff = 0
    for g, BPC in enumerate(A_CHUNKS):
        CH = 128 * BPC
        t0 = T + 128 * boff
        if g == 0:
            xh = xh0
            nc.sync.dma_start(out=xh0, in_=xht[:, :, 0 : T + CH])
            toff = T
        else:
            xh = xa.tile([128, DC, 128 * max(A_CHUNKS)], BF16, tag="xh")
            nc.sync.dma_start(out=xh[:, :, 0:CH], in_=xht[:, :, t0 : t0 + CH])
            toff = 0
        for j in range(BPC):
            blk = boff + j
            o = psum[:, blk, :]
            tok = slice(toff + 128 * j, toff + 128 * (j + 1))
            for c in range(DC):
                nc.tensor.matmul(
                    out=o, lhsT=xh[:, c, tok], rhs=sgh_sb[:, c, :],
                    start=(c == 0), stop=(c == DC - 1),
                )
        # stream this chunk's scores out on the Activation DMA queue
        sc = sb.tile([128, max(A_CHUNKS), T], F32, tag="sc")
        sc = sc[:, 0:BPC, :]
        nc.vector.tensor_copy(out=sc, in_=psum[:, boff : boff + BPC, :])
        nc.scalar.dma_start(out=scores[:, boff : boff + BPC, :], in_=sc)
        boff += BPC


def build_launch_b(iters=1):
    """Grouped GEMM over 17 pre-gathered 128-token blocks + exact rescore."""
    nc = bacc.Bacc(None)
    gxt = nc.dram_tensor("gxt", [128, DC, GCAP], BF16, kind="ExternalInput")
    wts = nc.dram_tensor("wts", [128, 2, DC, D], BF16, kind="ExternalInput")
    bts = nc.dram_tensor("bts", [1, 2 * D], BF16, kind="ExternalInput")
    rxt = nc.dram_tensor("rxt", [128, DC, T + RTOK], BF16, kind="ExternalInput")
    rlt = nc.dram_tensor("rlt", [128, DC, T + RTOK], F8, kind="ExternalInput")
    orows = nc.dram_tensor("orows", [GCAP, D], BF16, kind="ExternalOutput")
    rsc = nc.dram_tensor("rsc", [RTOK, T], F32, kind="ExternalOutput")

    with tile.TileContext(nc) as tc:
        with (
            tc.tile_pool(name="wp", bufs=1) as wp,
            tc.tile_pool(name="gx", bufs=3) as gxp,
            tc.tile_pool(name="ps", bufs=3, space="PSUM") as ps,
            tc.tile_pool(name="bp", bufs=2, space="PSUM") as bp,
            tc.tile_pool(name="osb", bufs=3) as osb,
        ):
            loop = tc.For_i(0, iters, 1) if iters > 1 else contextlib.nullcontext()
            with loop:
                _body_b(nc, wp, gxp, ps, bp, osb, gxt, wts, bts, rxt, rlt, orows, rsc)
    nc.compile()
    return nc


def _body_b(nc, wp, gxp, ps, bp, osb, gxt, wts, bts, rxt, rlt, orows, rsc):
    w_sb = wp.tile([128, 2, DC, D], BF16, tag="w")
    b_sb = wp.tile([128, 2, D], F32, tag="b")
    ones = wp.tile([1, 512], BF16, tag="ones")
    bts_sb = wp.tile([1, 2 * D], BF16, tag="btsb")
    rx_sb = wp.tile([128, DC, T + RTOK], BF16, tag="rx")
    rl_sb = wp.tile([128, DC, T + RTOK], F8, tag="rl")

    offs = np.cumsum([0] + GX_CHUNKS)
    gx_tiles = [None] * len(GX_CHUNKS)

    def emit_gx(ci):
        t = gxp.tile([128, DC, 512], BF16, tag="gx")
        n = GX_CHUNKS[ci] * 128
        nc.sync.dma_start(
            out=t[:, :, 0:n], in_=gxt[:, :, 128 * offs[ci] : 128 * offs[ci] + n]
        )
        gx_tiles[ci] = t

    def emit_bias():
        # bias broadcast via K=1 matmul (ones^T @ bias row) on the idle PE --
        # cheaper than a 128-partition DMA broadcast on the loaded DMA rails
        for s in range(2):
            for h in range(2):
                bps = bp.tile([128, 512], F32, tag="bps")
                nc.tensor.matmul(
                    out=bps, lhsT=ones[:, 0:128],
                    rhs=bts_sb[:, s * D + 512 * h : s * D + 512 * (h + 1)],
                    start=True, stop=True,
                )
                nc.vector.tensor_copy(out=b_sb[:, s, 512 * h : 512 * (h + 1)], in_=bps)

    def emit_rescore():
        # exact bf16hi+fp8lo rescore of this core's 64 borderline tokens
        rps = bp.tile([RTOK, T], F32, tag="bps")
        for c in range(DC):
            nc.tensor.matmul(
                out=rps, lhsT=rx_sb[:, c, T : T + RTOK], rhs=rx_sb[:, c, 0:T],
                start=(c == 0), stop=False,
            )
        for c in range(DC):
            nc.tensor.matmul(
                out=rps, lhsT=rl_sb[:, c, T : T + RTOK], rhs=rl_sb[:, c, 0:T],
                start=False, stop=(c == DC - 1),
            )
        rs = osb.tile([RTOK, T], F32, tag="rs")
        nc.vector.tensor_copy(out=rs, in_=rps)
        nc.scalar.dma_start(out=rsc[:, :], in_=rs)

    def drain(slot, ps0, ps1):
        slab = 0 if slot < RUN0 else 1
        o = osb.tile([128, D], BF16)
        nc.vector.tensor_add(out=o[:, 0:512], in0=ps0, in1=b_sb[:, slab, 0:512])
        nc.vector.tensor_add(out=o[:, 512:1024], in0=ps1, in1=b_sb[:, slab, 512:1024])
        nc.gpsimd.dma_start(out=orows[128 * slot : 128 * (slot + 1), :], in_=o)

    def compute_chunk0():
        # c-major over the first 2 slots: PE consumes one W chunk per 852ns
        # against the 728ns/chunk W stream, so the slab-0 load never stalls it
        t = gx_tiles[0]
        pses = []
        for si in range(GX_CHUNKS[0]):
            p0 = ps.tile([128, 512], F32, tag="ps0")
            p1 = ps.tile([128, 512], F32, tag="ps1")
            pses.append((p0, p1))
        for c in range(DC):
            for si in range(GX_CHUNKS[0]):
                p0, p1 = pses[si]
                tok = slice(128 * si, 128 * (si + 1))
                nc.tensor.matmul(
                    out=p0, lhsT=t[:, c, tok], rhs=w_sb[:, 0, c, 0:512],
                    start=(c == 0), stop=(c == DC - 1),
                )
                nc.tensor.matmul(
                    out=p1, lhsT=t[:, c, tok], rhs=w_sb[:, 0, c, 512:1024],
                    start=(c == 0), stop=(c == DC - 1),
                )
            if c == 0:
                emit_bias()
        for si in range(GX_CHUNKS[0]):
            drain(si, *pses[si])

    def compute_chunk(ci):
        t = gx_tiles[ci]
        for si in range(GX_CHUNKS[ci]):
            slot = offs[ci] + si
            slab = 0 if slot < RUN0 else 1
            tok = slice(128 * si, 128 * (si + 1))
            ps0 = ps.tile([128, 512], F32, tag="ps0")
            ps1 = ps.tile([128, 512], F32, tag="ps1")
            for c in range(DC):
                nc.tensor.matmul(
                    out=ps0, lhsT=t[:, c, tok], rhs=w_sb[:, slab, c, 0:512],
                    start=(c == 0), stop=(c == DC - 1),
                )
                nc.tensor.matmul(
                    out=ps1, lhsT=t[:, c, tok], rhs=w_sb[:, slab, c, 512:1024],
                    start=(c == 0), stop=(c == DC - 1),
                )
            drain(slot, ps0, ps1)

    # DMA emission order controls transfer order on the shared DMA engines:
    # tiny bias row, first W chunk, first gx chunk, rest of slab0, ...
    nc.vector.memset(ones, 1.0)
    nc.sync.dma_start(out=bts_sb, in_=bts[:, :])
    for i in range(NWARM):
        wps = ps.tile([128, 512], F32, tag="ps0" if i % 2 == 0 else "ps1")
        nc.tensor.matmul(out=wps, lhsT=ones[:, 0:128], rhs=ones[:, :], start=True, stop=True)
    nc.sync.dma_start(out=w_sb[:, 0, 0, :], in_=wts[:, 0, 0, :])
    emit_gx(0)
    for c in range(1, DC):
        nc.sync.dma_start(out=w_sb[:, 0, c, :], in_=wts[:, 0, c, :])
    emit_gx(1)
    compute_chunk0()
    for c in range(DC):
        nc.sync.dma_start(out=w_sb[:, 1, c, :], in_=wts[:, 1, c, :])
    nc.sync.dma_start(out=rx_sb, in_=rxt[:, :, :])
    nc.sync.dma_start(out=rl_sb, in_=rlt[:, :, :])
    emit_gx(2)
    compute_chunk(1)
    emit_rescore()
    emit_gx(3)
    compute_chunk(2)
    emit_gx(4)
    compute_chunk(3)
    compute_chunk(4)


_nc_a = None
_nc_b = None


def _get_programs():
    global _nc_a, _nc_b
    if _nc_a is None:
        _nc_a = build_launch_a()
        _nc_b = build_launch_b()
    return _nc_a, _nc_b


def _run_spmd(nc, in_maps, label):
    if os.environ.get("BASS_SIM"):
        from concourse.bass_interp import CoreSim

        results = []
        for im in in_maps:
            sim = CoreSim(nc)
            for k, v in im.items():
                sim.tensor(k)[:] = v
            sim.simulate()
            out = {}
            for alloc in nc.m.functions[0].allocations:
                if getattr(alloc, "kind", None) == "ExternalOutput":
                    name = alloc.memorylocations[0].name
                    out[name] = np.array(sim.mem_tensor(name))
            results.append(out)

        class R:
            pass

        r = R()
        r.results = results
        r.exec_time_ns = None
        return r
    from concourse.bass_utils import run_bass_kernel_spmd

    trace = bool(os.environ.get("BASS_TRACE"))
    kw = {}
    if trace:
        tdir = os.path.abspath(f"trace_{label}")
        os.makedirs(tdir, exist_ok=True)
        kw = dict(trace=True, tmpdir=tdir, trace_cores=[0])
    res = run_bass_kernel_spmd(nc, in_maps, core_ids=list(range(NCORES)), **kw)
    if trace:
        _perf.append((label, res.exec_time_ns, res.mean_exec_time_ns))
    return res


def _solve_runs(blocks_e, runs):
    """Cover each expert's block count with runs (core, slab, cap).

    Greedy: experts by descending need; prefer the largest run that fits
    exactly under the need, else burn the smallest run that overshoots.
    """
    runs = sorted(runs, key=lambda r: -r[2])
    assign = {e: [] for e in range(len(blocks_e))}
    need = {e: int(n) for e, n in enumerate(blocks_e)}
    for e in sorted(range(len(blocks_e)), key=lambda e: -blocks_e[e]):
        while need[e] > 0:
            fit = [r for r in runs if r[2] <= need[e]]
            if fit:
                r = fit[0]
            else:
                if not runs:
                    return None
                r = min(runs, key=lambda r: r[2])
            runs.remove(r)
            assign[e].append(r)
            need[e] -= r[2]
    return assign


def kernel(x, tile_sigs, W, b):
    x = np.asarray(x, np.float32)
    tile_sigs = np.asarray(tile_sigs, np.float32)
    W = np.asarray(W, np.float32)
    b = np.asarray(b, np.float32)
    _perf.clear()

    nc_a, nc_b = _get_programs()

    xf = x.reshape(NTOK, D)
    x_hi = xf.astype(BF16NP)
    sgnf = np.sign(tile_sigs).astype(np.float32)  # [T, D]
    # [p, c, t] layouts: element [p,c,t] = sgn[t, 128c+p]
    sgh = np.ascontiguousarray(
        sgnf.T.astype(BF16NP).reshape(DC, 128, T).transpose(1, 0, 2)
    )

    in_maps_a = []
    for c in range(NCORES):
        sh = slice(c * SHARD, (c + 1) * SHARD)
        # xht[p, ch, T+n] = x_hi[n, 128*ch + p]; sign vectors in cols 0..T
        xht = np.empty((128, DC, T + SHARD), BF16NP)
        xht[:, :, :T] = sgh
        xht[:, :, T:] = x_hi[sh].T.reshape(DC, 128, SHARD).transpose(1, 0, 2)
        in_maps_a.append({"xht": xht})

    res_a = _run_spmd(nc_a, in_maps_a, "a")
    # scores [128, ABLK, T]: token 128*j + p of the shard at [p, j, :]
    sa = np.concatenate(
        [
            np.asarray(res_a.results[c]["scores"], np.float32)
            .transpose(1, 0, 2).reshape(SHARD, T)
            for c in range(NCORES)
        ]
    )  # [NTOK, T] approx scores

    # candidate sets: every expert within THETA of the approx max
    smax = sa.max(1)
    cands = sa > (smax - THETA)[:, None]
    ncand = cands.sum(1)
    a1 = sa.argmax(1)
    border = np.nonzero(ncand > 1)[0]
    assert len(border) <= NCORES * RTOK, f"too many borderline tokens: {len(border)}"

    # expert token lists (borderline tokens in every candidate list)
    lists = []
    for t in range(T):
        tl = np.nonzero(cands[:, t] & ((ncand > 1) | (a1 == t)))[0]
        lists.append(tl)
    blocks_e = [int(np.ceil(len(tl) / 128)) for tl in lists]
    assert sum(blocks_e) <= NCORES * NSLOT, f"capacity exceeded: {blocks_e}"
    runs = [(c, 0, RUN0) for c in range(NCORES)] + [(c, 1, RUN1) for c in range(NCORES)]
    assign = _solve_runs(blocks_e, runs)
    assert assign is not None, f"block assignment infeasible for blocks {blocks_e}"

    slot_expert = np.zeros((NCORES, 2), np.int64)
    slot_tokens = np.full((NCORES, GCAP), TRASH, np.int64)
    for t in range(T):
        ids = lists[t]
        pos = 0
        for core, sl, cap in assign[t]:
            slot_expert[core, sl] = t
            base = 0 if sl == 0 else RUN0 * 128
            take = ids[pos : pos + cap * 128]
            slot_tokens[core, base : base + len(take)] = take
            pos += len(take)
        assert pos == len(ids)

    # launch B inputs
    x_pad = np.vstack([x_hi, np.zeros((1, D), BF16NP)])  # [NTOK+1, D]
    x_lo8 = ((xf - x_hi.astype(np.float32)) * 64.0).astype(F8NP)
    xl_pad = np.vstack([x_lo8, np.zeros((1, D), F8NP)])
    sgl = np.ascontiguousarray(
        (sgnf.T / 64.0).astype(F8NP).reshape(DC, 128, T).transpose(1, 0, 2)
    )
    # Wb[t, p, ch, e] = W[t, 128*ch + p, e]
    Wb = np.ascontiguousarray(
        W.astype(BF16NP).reshape(T, DC, 128, D).transpose(0, 2, 1, 3)
    )
    rtok_all = np.full(NCORES * RTOK, TRASH, np.int64)
    rtok_all[: len(border)] = border
    in_maps_b = []
    for core in range(NCORES):
        ids = slot_tokens[core]
        rows = x_pad[ids]  # [GCAP, D] bf16
        gxt = np.ascontiguousarray(rows.reshape(GCAP, DC, 128).transpose(2, 1, 0))
        wts = np.ascontiguousarray(
            np.stack([Wb[slot_expert[core, 0]], Wb[slot_expert[core, 1]]], axis=1)
        )  # [128, 2, DC, D]
        bts = np.ascontiguousarray(
            np.concatenate([b[slot_expert[core, 0]], b[slot_expert[core, 1]]])
            .astype(BF16NP).reshape(1, 2 * D)
        )  # [1, 2*D] bf16
        rids = rtok_all[core * RTOK : (core + 1) * RTOK]
        rxt = np.empty((128, DC, T + RTOK), BF16NP)
        rxt[:, :, :T] = sgh
        rxt[:, :, T:] = x_pad[rids].reshape(RTOK, DC, 128).transpose(2, 1, 0)
        rlt = np.empty((128, DC, T + RTOK), F8NP)
        rlt[:, :, :T] = sgl
        rlt[:, :, T:] = xl_pad[rids].reshape(RTOK, DC, 128).transpose(2, 1, 0)
        in_maps_b.append({"gxt": gxt, "wts": wts, "bts": bts, "rxt": rxt, "rlt": rlt})

    res_b = _run_spmd(nc_b, in_maps_b, "b")

    # final routing decision: rescored argmax for borderline tokens
    chosen = a1.copy()
    if len(border):
        rsc = np.concatenate(
            [np.asarray(res_b.results[c]["rsc"], np.float32) for c in range(NCORES)]
        )[: len(border)]
        rpick = rsc.argmax(1)
        ok = cands[border, rpick]
        assert ok.all(), "rescored argmax outside candidate set"
        chosen[border] = rpick

    out_pad = np.zeros((NTOK, D), np.float32)
    for core in range(NCORES):
        orows = np.asarray(res_b.results[core]["orows"]).astype(np.float32)
        ids = slot_tokens[core]
        exp_of_slot = np.where(np.arange(GCAP) < RUN0 * 128,
                               slot_expert[core, 0], slot_expert[core, 1])
        valid = (ids < NTOK) & (chosen[np.minimum(ids, NTOK - 1)] == exp_of_slot)
        out_pad[ids[valid]] = orows[valid]
    return out_pad.reshape(B, S, D)


# revision 32
# speedup vs baseline: 1.1808x; 1.0360x over previous
"""ContentOnlyRouter MoE kernel for 8x TRN2 NeuronCores.

Strategy (two SPMD launches, host does only data marshalling/selection):
  Launch A (data-parallel approx scoring): each core scores its 2048-token
    shard against sign(tile_sigs) in bf16 only (half the DMA of an exact
    hi/lo split) and ships the fp32 scores. bf16 scoring has a bounded
    absolute error (<0.27 on this input distribution), so any token whose
    top-2 approx gap exceeds THETA=0.53 > 2*err_max provably has the true
    argmax; the rest ("borderline", ~400 of 16384) are routed to EVERY
    candidate expert within THETA and disambiguated by launch B's exact
    rescore. Sign vectors ride in the first T columns of chunk 0; scores
    stream out per 512-token chunk on the Activation DMA queue.
  Host glue: candidate sets from approx scores; expert token lists padded
    to 128-multiples; blocks packed onto 8 cores x 17 block-slots (slots
    0-8 = weight slab 0, 9-16 = slab 1) by a greedy covering solver. The
    gather (pick + transpose token rows) happens on host.
  Launch B (block-parallel grouped GEMM + rescore): each core streams its
    17 pre-gathered 128-token blocks and 2 weight slabs; 8 accumulating
    bf16 matmuls per 512-wide PSUM half; bias is built by a K=1 matmul on
    the idle PE (ones x bias-row broadcast) and added on DVE; bf16 rows
    out. A PE warm-up (dep-free matmuls on a constant tile, with the bias
    matmuls slotted in) burns the p-state ramp before the GEMM so every
    GEMM matmul runs at full clock. Each core also rescores its 64-token
    share of the borderline set exactly (bf16 hi + fp8e4m3 lo, lo scaled
    by 64 with sign vectors scaled by 1/64 -- products exact, fp32 PSUM
    accumulation; verified argmax-exact on this distribution). Host keeps,
    per borderline token, the row computed under the rescored-argmax
    expert.

Shapes hardcoded for B=4, S=4096, D=1024, T=8 per the problem spec.
"""

import os

os.environ.setdefault("JAX_PLATFORMS", "")

import contextlib

import numpy as np
import ml_dtypes

import concourse.bass as bass
import concourse.bacc as bacc
import concourse.mybir as mybir
import concourse.tile as tile

B, S, D, T = 4, 4096, 1024, 8
NTOK = B * S             # 16384 tokens
NCORES = 8
SHARD = NTOK // NCORES   # 2048 tokens scored per core
DC = D // 128            # 8 contraction chunks
ABLK = SHARD // 128      # 16 token blocks per shard
A_CHUNKS = [4, 4, 4, 4]  # launch A DMA chunk sizes in 128-token blocks
A_WARMS = [0, 0, 0, 0]   # A is DMA-bound; PE warms only add SEQ decode traffic
NSLOT = 17               # GEMM block slots per core
RUN0, RUN1 = 9, 8        # slots per weight slab (slab0: slots 0-8, slab1: 9-16)
GCAP = NSLOT * 128       # 2176 gathered tokens per core
TRASH = NTOK             # row index used for padding slots
GX_CHUNKS = [2, 1, 2, 4, 4, 4]  # slots per launch-B gather-stream chunk
THETA = 0.53             # borderline gap threshold (> 2*max bf16 score err)
RTOK = 64                # borderline tokens rescored per core (512 total)
NWARM = int(os.environ.get('NWARM', 40))                # PE warm-up matmuls: burn the p-state ramp pre-GEMM

F32 = mybir.dt.float32
BF16 = mybir.dt.bfloat16
F8 = mybir.dt.float8e4

BF16NP = ml_dtypes.bfloat16
F8NP = ml_dtypes.float8_e4m3

_perf = []  # exec_time_ns per launch when tracing


def build_launch_a(iters=1):
    """bf16 approx scores for one 2048-token shard."""
    nc = bacc.Bacc(None)
    xht = nc.dram_tensor("xht", [128, DC, T + SHARD], BF16, kind="ExternalInput")
    scores = nc.dram_tensor("scores", [128, ABLK, T], F32, kind="ExternalOutput")

    with tile.TileContext(nc) as tc:
        with (
            tc.tile_pool(name="x0", bufs=1) as x0p,
            tc.tile_pool(name="xa", bufs=3) as xa,
            tc.tile_pool(name="ps", bufs=1, space="PSUM") as ps,
            tc.tile_pool(name="sb", bufs=4) as sb,
        ):
            loop = tc.For_i(0, iters, 1) if iters > 1 else contextlib.nullcontext()
            with loop:
                _body_a(nc, x0p, xa, ps, sb, xht, scores)
    nc.compile()
    return nc


def _body_a(nc, x0p, xa, ps, sb, xht, scores):
    psum = ps.tile([128, ABLK, T], F32)
    wps = ps.tile([128, 512], F32, tag="wps")
    ones_a = x0p.tile([1, 128], BF16, tag="ones_a")
    nc.vector.memset(ones_a, 1.0)

    def warm(n):
        for _ in range(n):
            nc.tensor.matmul(out=wps[:, 0:128], lhsT=ones_a[:, :], rhs=ones_a[:, :],
                             start=True, stop=True)
    CH0 = 128 * A_CHUNKS[0]
    xh0 = x0p.tile([128, DC, T + CH0], BF16, tag="xh0")
    sgh_sb = xh0[:, :, 0:T]
    boff = 0
    for g, BPC in enumerate(A_CHUNKS):
        CH = 128 * BPC
        t0 = T + 128 * boff
        if g == 0:
            xh = xh0
            nc.sync.dma_start(out=xh0, in_=xht[:, :, 0 : T + CH])
            toff = T
        else:
            xh = xa.tile([128, DC, 128 * max(A_CHUNKS)], BF16, tag="xh")
            nc.sync.dma_start(out=xh[:, :, 0:CH], in_=xht[:, :, t0 : t0 + CH])
            toff = 0
        warm(A_WARMS[g])
        for j in range(BPC):
            blk = boff + j
            o = psum[:, blk, :]
            tok = slice(toff + 128 * j, toff + 128 * (j + 1))
            for c in range(DC):
                nc.tensor.matmul(
                    out=o, lhsT=xh[:, c, tok], rhs=sgh_sb[:, c, :],
                    start=(c == 0), stop=(c == DC - 1),
                )
        # stream this chunk's scores out on the Activation DMA queue
        sc = sb.tile([128, max(A_CHUNKS), T], F32, tag="sc")
        sc = sc[:, 0:BPC, :]
        nc.vector.tensor_copy(out=sc, in_=psum[:, boff : boff + BPC, :])
        nc.scalar.dma_start(out=scores[:, boff : boff + BPC, :], in_=sc)
        boff += BPC


def build_launch_b(iters=1):
    """Grouped GEMM over 17 pre-gathered 128-token blocks + exact rescore."""
    nc = bacc.Bacc(None)
    gxt = nc.dram_tensor("gxt", [128, DC, GCAP], BF16, kind="ExternalInput")
    wts = nc.dram_tensor("wts", [128, 2, DC, D], BF16, kind="ExternalInput")
    bts = nc.dram_tensor("bts", [1, 2 * D], BF16, kind="ExternalInput")
    rxt = nc.dram_tensor("rxt", [128, DC, T + RTOK], BF16, kind="ExternalInput")
    rlt = nc.dram_tensor("rlt", [128, DC, T + RTOK], F8, kind="ExternalInput")
    orows = nc.dram_tensor("orows", [GCAP, D], BF16, kind="ExternalOutput")
    rsc = nc.dram_tensor("rsc", [RTOK, T], F32, kind="ExternalOutput")

    with tile.TileContext(nc) as tc:
        with (
            tc.tile_pool(name="wp", bufs=1) as wp,
            tc.tile_pool(name="gx", bufs=3) as gxp,
            tc.tile_pool(name="ps", bufs=3, space="PSUM") as ps,
            tc.tile_pool(name="bp", bufs=2, space="PSUM") as bp,
            tc.tile_pool(name="osb", bufs=3) as osb,
        ):
            loop = tc.For_i(0, iters, 1) if iters > 1 else contextlib.nullcontext()
            with loop:
                _body_b(nc, wp, gxp, ps, bp, osb, gxt, wts, bts, rxt, rlt, orows, rsc)
    nc.compile()
    return nc


def _body_b(nc, wp, gxp, ps, bp, osb, gxt, wts, bts, rxt, rlt, orows, rsc):
    w_sb = wp.tile([128, 2, DC, D], BF16, tag="w")
    b_sb = wp.tile([128, 2, D], F32, tag="b")
    ones = wp.tile([1, 128], BF16, tag="ones")
    bts_sb = wp.tile([1, 2 * D], BF16, tag="btsb")
    rx_sb = wp.tile([128, DC, T + RTOK], BF16, tag="rx")
    rl_sb = wp.tile([128, DC, T + RTOK], F8, tag="rl")

    offs = np.cumsum([0] + GX_CHUNKS)
    gx_tiles = [None] * len(GX_CHUNKS)

    def emit_gx(ci):
        t = gxp.tile([128, DC, 512], BF16, tag="gx")
        n = GX_CHUNKS[ci] * 128
        o0 = 128 * offs[ci]
        nc.sync.dma_start(out=t[:, :, 0:n], in_=gxt[:, :, o0 : o0 + n])
        gx_tiles[ci] = t

    def emit_gx0_halves():
        # chunk 0 as two separate tiles (c 0-3, c 4-7): tile-granular DMA
        # deps let the first c-passes start once half the data has landed
        h = DC // 2
        n = GX_CHUNKS[0] * 128
        ta = gxp.tile([128, h, 256], BF16, tag="gx0a")
        nc.sync.dma_start(out=ta[:, :, 0:n], in_=gxt[:, 0:h, 0:n])
        nc.sync.dma_start(out=w_sb[:, 0, 1, :], in_=wts[:, 0, 1, :])
        nc.sync.dma_start(out=w_sb[:, 0, 2, :], in_=wts[:, 0, 2, :])
        tb = gxp.tile([128, h, 256], BF16, tag="gx0b")
        nc.sync.dma_start(out=tb[:, :, 0:n], in_=gxt[:, h:DC, 0:n])
        gx_tiles[0] = (ta, tb)

    def emit_rescore():
        # exact bf16hi+fp8lo rescore of this core's 64 borderline tokens
        rps = bp.tile([RTOK, T], F32, tag="bps")
        for c in range(DC):
            nc.tensor.matmul(
                out=rps, lhsT=rx_sb[:, c, T : T + RTOK], rhs=rx_sb[:, c, 0:T],
                start=(c == 0), stop=False,
            )
        for c in range(DC):
            nc.tensor.matmul(
                out=rps, lhsT=rl_sb[:, c, T : T + RTOK], rhs=rl_sb[:, c, 0:T],
                start=False, stop=(c == DC - 1),
            )
        rs = osb.tile([RTOK, T], F32, tag="rs")
        nc.vector.tensor_copy(out=rs, in_=rps)
        nc.scalar.dma_start(out=rsc[:, :], in_=rs)

    def drain(slot, ps0, ps1):
        slab = 0 if slot < RUN0 else 1
        o = osb.tile([128, D], BF16)
        nc.vector.tensor_add(out=o[:, 0:512], in0=ps0, in1=b_sb[:, slab, 0:512])
        nc.vector.tensor_add(out=o[:, 512:1024], in0=ps1, in1=b_sb[:, slab, 512:1024])
        # the final slot's write rides the idle ACT HWDGE queue: cheaper
        # dispatch than Pool's SWDGE on the end-of-launch critical path
        eng = nc.scalar if slot == NSLOT - 1 else nc.gpsimd
        eng.dma_start(out=orows[128 * slot : 128 * (slot + 1), :], in_=o)

    def compute_chunk0():
        # c-major over the first 2 slots: PE consumes one W chunk per 852ns
        # against the 728ns/chunk W stream, so the slab-0 load never stalls it
        ta, tb = gx_tiles[0]
        h = DC // 2
        pses = []
        for si in range(GX_CHUNKS[0]):
            p0 = ps.tile([128, 512], F32, tag="ps0")
            p1 = ps.tile([128, 512], F32, tag="ps1")
            pses.append((p0, p1))
        for c in range(DC):
            t = ta if c < h else tb
            cc = c if c < h else c - h
            for si in range(GX_CHUNKS[0]):
                p0, p1 = pses[si]
                tok = slice(128 * si, 128 * (si + 1))
                nc.tensor.matmul(
                    out=p0, lhsT=t[:, cc, tok], rhs=w_sb[:, 0, c, 0:512],
                    start=(c == 0), stop=(c == DC - 1),
                )
                nc.tensor.matmul(
                    out=p1, lhsT=t[:, cc, tok], rhs=w_sb[:, 0, c, 512:1024],
                    start=(c == 0), stop=(c == DC - 1),
                )
        for si in range(GX_CHUNKS[0]):
            drain(si, *pses[si])

    def compute_chunk(ci):
        t = gx_tiles[ci]
        for si in range(GX_CHUNKS[ci]):
            slot = offs[ci] + si
            slab = 0 if slot < RUN0 else 1
            tok = slice(128 * si, 128 * (si + 1))
            last = slot == NSLOT - 1
            ps0 = ps.tile([128, 512], F32, tag="ps0")
            ps1 = ps.tile([128, 512], F32, tag="ps1")
            if last:
                # preload bias into PSUM so the final drain is a plain copy
                # that can split across DVE and ACT in parallel -- shortens
                # the end-of-launch critical chain by ~0.6us
                nc.vector.tensor_copy(out=ps0, in_=b_sb[:, slab, 0:512])
                nc.vector.tensor_copy(out=ps1, in_=b_sb[:, slab, 512:1024])
            for c in range(DC):
                nc.tensor.matmul(
                    out=ps0, lhsT=t[:, c, tok], rhs=w_sb[:, slab, c, 0:512],
                    start=(c == 0) and not last, stop=(c == DC - 1),
                    skip_group_check=last,
                )
                nc.tensor.matmul(
                    out=ps1, lhsT=t[:, c, tok], rhs=w_sb[:, slab, c, 512:1024],
                    start=(c == 0) and not last, stop=(c == DC - 1),
                    skip_group_check=last,
                )
            if last:
                o = osb.tile([128, D], BF16)
                nc.vector.tensor_copy(out=o[:, 0:512], in_=ps0)
                nc.scalar.copy(out=o[:, 512:1024], in_=ps1)
                nc.scalar.dma_start(out=orows[128 * slot : 128 * (slot + 1), :], in_=o)
            else:
                drain(slot, ps0, ps1)

    # DMA emission order controls transfer order on the shared DMA engines:
    # tiny bias row, first W chunk, first gx chunk, rest of slab0, ...
    nc.vector.memset(ones, 1.0)
    # bts rides the Pool/SWDGE queue: keeps the serialized HWDGE generation
    # slots on SP for the W/gx stream only
    nc.gpsimd.dma_start(out=bts_sb, in_=bts[:, :])
    bias_jobs = [(s, h) for s in range(2) for h in range(2)]
    bi = 0
    for i in range(NWARM):
        wps = ps.tile([128, 512], F32, tag="ps0" if i % 2 == 0 else "ps1")
        nc.tensor.matmul(out=wps[:, 0:128], lhsT=ones[:, :], rhs=ones[:, :], start=True, stop=True)
        # slot the 4 bias matmuls into the warm stream once bts has landed,
        # spaced so each DVE copy-back finishes before its bank is reused
        if i >= NWARM_BTS and bi < 4 and (i - NWARM_BTS) % 2 == 0:
            s, h = bias_jobs[bi]
            bi += 1
            bps = bp.tile([128, 512], F32, tag="bps")
            nc.tensor.matmul(
                out=bps, lhsT=ones[:, 0:128],
                rhs=bts_sb[:, s * D + 512 * h : s * D + 512 * (h + 1)],
                start=True, stop=True,
            )
            nc.vector.tensor_copy(out=b_sb[:, s, 512 * h : 512 * (h + 1)], in_=bps)
    assert bi == 4
    nc.sync.dma_start(out=w_sb[:, 0, 0, :], in_=wts[:, 0, 0, :])
    emit_gx0_halves()
    for c in range(3, DC):
        nc.sync.dma_start(out=w_sb[:, 0, c, :], in_=wts[:, 0, c, :])
        if c == 4:
            emit_gx(1)
    emit_gx(2)
    compute_chunk0()
    for c in range(DC):
        nc.sync.dma_start(out=w_sb[:, 1, c, :], in_=wts[:, 1, c, :])
    emit_gx(3)
    nc.sync.dma_start(out=rx_sb, in_=rxt[:, :, :])
    nc.sync.dma_start(out=rl_sb, in_=rlt[:, :, :])
    compute_chunk(1)
    compute_chunk(2)
    emit_gx(4)
    compute_chunk(3)
    emit_rescore()
    emit_gx(5)
    compute_chunk(4)
    compute_chunk(5)


_nc_a = None
_nc_b = None


def _get_programs():
    global _nc_a, _nc_b
    if _nc_a is None:
        _nc_a = build_launch_a()
        _nc_b = build_launch_b()
    return _nc_a, _nc_b


def _run_spmd(nc, in_maps, label):
    if os.environ.get("BASS_SIM"):
        from concourse.bass_interp import CoreSim

        results = []
        for im in in_maps:
            sim = CoreSim(nc)
            for k, v in im.items():
                sim.tensor(k)[:] = v
            sim.simulate()
            out = {}
            for alloc in nc.m.functions[0].allocations:
                if getattr(alloc, "kind", None) == "ExternalOutput":
                    name = alloc.memorylocations[0].name
                    out[name] = np.array(sim.mem_tensor(name))
            results.append(out)

        class R:
            pass

        r = R()
        r.results = results
        r.exec_time_ns = None
        return r
    from concourse.bass_utils import run_bass_kernel_spmd

    trace = bool(os.environ.get("BASS_TRACE"))
    kw = {}
    if trace:
        tdir = os.path.abspath(f"trace_{label}")
        os.makedirs(tdir, exist_ok=True)
        kw = dict(trace=True, tmpdir=tdir, trace_cores=[0])
    res = run_bass_kernel_spmd(nc, in_maps, core_ids=list(range(NCORES)), **kw)
    if trace:
        _perf.append((label, res.exec_time_ns, res.mean_exec_time_ns))
    return res


def _solve_runs(blocks_e, runs):
    """Cover each expert's block count with runs (core, slab, cap).

    Greedy: experts by descending need; prefer the largest run that fits
    exactly under the need, else burn the smallest run that overshoots.
    """
    runs = sorted(runs, key=lambda r: -r[2])
    assign = {e: [] for e in range(len(blocks_e))}
    need = {e: int(n) for e, n in enumerate(blocks_e)}
    for e in sorted(range(len(blocks_e)), key=lambda e: -blocks_e[e]):
        while need[e] > 0:
            fit = [r for r in runs if r[2] <= need[e]]
            if fit:
                r = fit[0]
            else:
                if not runs:
                    return None
                r = min(runs, key=lambda r: r[2])
            runs.remove(r)
            assign[e].append(r)
            need[e] -= r[2]
    return assign


def kernel(x, tile_sigs, W, b):
    x = np.asarray(x, np.float32)
    tile_sigs = np.asarray(tile_sigs, np.float32)
    W = np.asarray(W, np.float32)
    b = np.asarray(b, np.float32)
    _perf.clear()

    nc_a, nc_b = _get_programs()

    xf = x.reshape(NTOK, D)
    x_hi = xf.astype(BF16NP)
    sgnf = np.sign(tile_sigs).astype(np.float32)  # [T, D]
    # [p, c, t] layouts: element [p,c,t] = sgn[t, 128c+p]
    sgh = np.ascontiguousarray(
        sgnf.T.astype(BF16NP).reshape(DC, 128, T).transpose(1, 0, 2)
    )

    in_maps_a = []
    for c in range(NCORES):
        sh = slice(c * SHARD, (c + 1) * SHARD)
        # xht[p, ch, T+n] = x_hi[n, 128*ch + p]; sign vectors in cols 0..T
        xht = np.empty((128, DC, T + SHARD), BF16NP)
        xht[:, :, :T] = sgh
        xht[:, :, T:] = x_hi[sh].T.reshape(DC, 128, SHARD).transpose(1, 0, 2)
        in_maps_a.append({"xht": xht})

    res_a = _run_spmd(nc_a, in_maps_a, "a")
    # scores [128, ABLK, T]: token 128*j + p of the shard at [p, j, :]
    sa = np.concatenate(
        [
            np.asarray(res_a.results[c]["scores"], np.float32)
            .reshape(128, ABLK, T).transpose(1, 0, 2).reshape(SHARD, T)
            for c in range(NCORES)
        ]
    )  # [NTOK, T] approx scores

    # candidate sets: every expert within THETA of the approx max
    smax = sa.max(1)
    cands = sa > (smax - THETA)[:, None]
    ncand = cands.sum(1)
    a1 = sa.argmax(1)
    border = np.nonzero(ncand > 1)[0]
    assert len(border) <= NCORES * RTOK, f"too many borderline tokens: {len(border)}"

    # expert token lists (borderline tokens in every candidate list)
    lists = []
    for t in range(T):
        tl = np.nonzero(cands[:, t] & ((ncand > 1) | (a1 == t)))[0]
        lists.append(tl)
    blocks_e = [int(np.ceil(len(tl) / 128)) for tl in lists]
    assert sum(blocks_e) <= NCORES * NSLOT, f"capacity exceeded: {blocks_e}"
    runs = [(c, 0, RUN0) for c in range(NCORES)] + [(c, 1, RUN1) for c in range(NCORES)]
    assign = _solve_runs(blocks_e, runs)
    assert assign is not None, f"block assignment infeasible for blocks {blocks_e}"

    slot_expert = np.zeros((NCORES, 2), np.int64)
    slot_tokens = np.full((NCORES, GCAP), TRASH, np.int64)
    for t in range(T):
        ids = lists[t]
        pos = 0
        for core, sl, cap in assign[t]:
            slot_expert[core, sl] = t
            base = 0 if sl == 0 else RUN0 * 128
            take = ids[pos : pos + cap * 128]
            slot_tokens[core, base : base + len(take)] = take
            pos += len(take)
        assert pos == len(ids)

    # launch B inputs
    x_pad = np.vstack([x_hi, np.zeros((1, D), BF16NP)])  # [NTOK+1, D]
    x_lo8 = ((xf - x_hi.astype(np.float32)) * 64.0).astype(F8NP)
    xl_pad = np.vstack([x_lo8, np.zeros((1, D), F8NP)])
    sgl = np.ascontiguousarray(
        (sgnf.T / 64.0).astype(F8NP).reshape(DC, 128, T).transpose(1, 0, 2)
    )
    # Wb[t, p, ch, e] = W[t, 128*ch + p, e]
    Wb = np.ascontiguousarray(
        W.astype(BF16NP).reshape(T, DC, 128, D).transpose(0, 2, 1, 3)
    )
    rtok_all = np.full(NCORES * RTOK, TRASH, np.int64)
    rtok_all[: len(border)] = border
    in_maps_b = []
    for core in range(NCORES):
        ids = slot_tokens[core]
        rows = x_pad[ids]  # [GCAP, D] bf16
        gxt = np.ascontiguousarray(rows.reshape(GCAP, DC, 128).transpose(2, 1, 0))
        wts = np.ascontiguousarray(
            np.stack([Wb[slot_expert[core, 0]], Wb[slot_expert[core, 1]]], axis=1)
        )  # [128, 2, DC, D]
        bts = np.ascontiguousarray(
            np.concatenate([b[slot_expert[core, 0]], b[slot_expert[core, 1]]])
            .astype(BF16NP).reshape(1, 2 * D)
        )  # [1, 2*D] bf16
        rids = rtok_all[core * RTOK : (core + 1) * RTOK]
        rxt = np.empty((128, DC, T + RTOK), BF16NP)
        rxt[:, :, :T] = sgh
        rxt[:, :, T:] = x_pad[rids].reshape(RTOK, DC, 128).transpose(2, 1, 0)
        rlt = np.empty((128, DC, T + RTOK), F8NP)
        rlt[:, :, :T] = sgl
        rlt[:, :, T:] = xl_pad[rids].reshape(RTOK, DC, 128).transpose(2, 1, 0)
        in_maps_b.append({"gxt": gxt, "wts": wts, "bts": bts, "rxt": rxt, "rlt": rlt})

    res_b = _run_spmd(nc_b, in_maps_b, "b")

    # final routing decision: rescored argmax for borderline tokens
    chosen = a1.copy()
    if len(border):
        rsc = np.concatenate(
            [np.asarray(res_b.results[c]["rsc"], np.float32).reshape(RTOK, T)
             for c in range(NCORES)]
        )[: len(border)]
        rpick = rsc.argmax(1)
        ok = cands[border, rpick]
        assert ok.all(), "rescored argmax outside candidate set"
        chosen[border] = rpick

    out_pad = np.zeros((NTOK, D), np.float32)
    for core in range(NCORES):
        orows = np.asarray(res_b.results[core]["orows"]).reshape(GCAP, D).astype(np.float32)
        ids = slot_tokens[core]
        exp_of_slot = np.where(np.arange(GCAP) < RUN0 * 128,
                               slot_expert[core, 0], slot_expert[core, 1])
        valid = (ids < NTOK) & (chosen[np.minimum(ids, NTOK - 1)] == exp_of_slot)
        out_pad[ids[valid]] = orows[valid]
    return out_pad.reshape(B, S, D)
